# revision 1
# baseline (speedup 1.0000x reference)
"""Bass/Trainium2 kernel for nn_EvoBinarizedLayer.

Reference computation (P=16 populations, B=512, I=O=2048, all values 0/1):
    out[p,b,o] = sum_i x[p,b,i]*w0[p,i,o] + (1-x[p,b,i])*w1[p,i,o]

Strategy:
  - Shard population dim P across 8 cores (2 pops/core), embarrassingly parallel.
  - Cast x/w to fp8e4m3 on host (0/1 values are exact); compute notx = 1-x on
    device (ACT/DVE); accumulate x@w0 + notx@w1 into the same PSUM bank via a
    single K=4096 "concat" contraction -> one accumulation group, no bias pass.
  - fp8 DoubleRow matmuls (K=256 per MM) for 2x PE throughput.
  - PSUM f32 accumulation of 0/1 products is exact (max 4096 < 2^24), so the
    result is bit-exact vs the f32 reference.

Host-side work is layout only: slicing, transpose, dtype cast, and the final
gather. All arithmetic (notx, matmuls) happens on device.
"""

import os

import numpy as np
import ml_dtypes

from concourse import bacc, tile, mybir
from concourse.bass_utils import run_bass_kernel_spmd

P_TOT, B, I, O = 16, 512, 2048, 2048
N_CORES = 8
PPC = P_TOT // N_CORES  # pops per core = 2
PART = 128

FP8 = mybir.dt.float8e4
F32 = mybir.dt.float32
NP_FP8 = ml_dtypes.float8_e4m3


def build_nc(ppc=PPC, b=B, i_dim=I, o_dim=O, n_cores=N_CORES, use_dr=True):
    """Build + compile the per-core Bass program (SPMD: same program, 8 cores)."""
    kt = i_dim // PART          # k-subtiles per weight tensor (16)
    nb = o_dim // 512           # o-blocks (4)
    mb = b // PART              # b-subtiles (4)
    DR = mybir.MatmulPerfMode.DoubleRow if use_dr else None
    kstep = 2 if use_dr else 1

    nc = bacc.Bacc("TRN2", target_bir_lowering=False, debug=False,
                   num_devices=n_cores)

    xt_d = nc.dram_tensor("xt", [ppc, PART, kt, b], FP8, kind="ExternalInput")
    w0_d = nc.dram_tensor("w0", [ppc, nb, PART, kt, 512], FP8, kind="ExternalInput")
    w1_d = nc.dram_tensor("w1", [ppc, nb, PART, kt, 512], FP8, kind="ExternalInput")
    out_d = nc.dram_tensor("out", [ppc, b, o_dim], F32, kind="ExternalOutput")

    with tile.TileContext(nc) as tc:
        with (
            tc.tile_pool(name="warm", bufs=1) as warm,
            tc.tile_pool(name="xpool", bufs=2) as xpool,
            tc.tile_pool(name="wpool", bufs=8) as wpool,
            tc.tile_pool(name="opool", bufs=4) as opool,
            tc.tile_pool(name="pspool", bufs=4, space="PSUM") as pspool,
            tc.tile_pool(name="warmps", bufs=1, space="PSUM") as warmps,
        ):
            for pop in range(ppc):
                xt = xpool.tile([PART, kt, b], FP8, tag="xt")
                nxt = xpool.tile([PART, kt, b], FP8, tag="nxt")
                # x chunked on the scalar ring ahead of w1: the first matmul
                # needs only xt[:, 0:2, :], so a 256KB first chunk unblocks
                # the first LDWEIGHTS ~10us sooner than one 1MB transfer.
                xch = min(4, kt)
                for ch in range(0, kt, xch):
                    nc.scalar.dma_start(out=xt[:, ch:ch + xch, :],
                                        in_=xt_d.ap()[pop, :, ch:ch + xch, :])
                    # notx = 1 - x  ==  (x * -1) + 1, per chunk
                    nc.vector.tensor_scalar(
                        nxt[:, ch:ch + xch, :], xt[:, ch:ch + xch, :], -1.0, 1.0,
                        mybir.AluOpType.mult, mybir.AluOpType.add,
                    )
                for nbi in range(nb):
                    w0t = wpool.tile([PART, kt, 512], FP8, tag="w")
                    w1t = wpool.tile([PART, kt, 512], FP8, tag="w")
                    # w0 loads on the sync HWDGE ring, w1 on the scalar HWDGE
                    # ring (output stores go via gpsimd/SWDGE) so stores never
                    # block weight prefetch in a shared FIFO. Chunked k-wise so
                    # the first matmuls start before the whole block lands; the
                    # very first block uses finer chunks to cut the startup
                    # bubble before the first LDWEIGHTS.
                    wch = 2 if (pop == 0 and nbi == 0) else 4
                    for ch in range(0, kt, wch):
                        nc.sync.dma_start(
                            out=w0t[:, ch:ch + wch, :],
                            in_=w0_d.ap()[pop, nbi, :, ch:ch + wch, :])
                        nc.scalar.dma_start(
                            out=w1t[:, ch:ch + wch, :],
                            in_=w1_d.ap()[pop, nbi, :, ch:ch + wch, :])
                    for m in range(mb):
                        ps = pspool.tile([PART, 512], F32)
                        msl = slice(m * PART, (m + 1) * PART)
                        nk = kt // kstep
                        for kd in range(nk):
                            ksl = slice(kd * kstep, (kd + 1) * kstep)
                            nc.tensor.matmul(
                                ps[:], lhsT=xt[:, ksl, msl], rhs=w0t[:, ksl, :],
                                start=(kd == 0), stop=False, perf_mode=DR,
                            )
                        for kd in range(nk):
                            ksl = slice(kd * kstep, (kd + 1) * kstep)
                            nc.tensor.matmul(
                                ps[:], lhsT=nxt[:, ksl, msl], rhs=w1t[:, ksl, :],
                                start=False, stop=(kd == nk - 1), perf_mode=DR,
                            )
                        ot = opool.tile([PART, 512], F32)
                        nc.vector.tensor_copy(ot[:], ps[:])
                        nc.gpsimd.dma_start(
                            out=out_d.ap()[pop, msl, nbi * 512:(nbi + 1) * 512],
                            in_=ot[:],
                        )
    nc.compile()
    return nc


def build_nc_v3(ppc=PPC, b=B, i_dim=I, o_dim=O, n_cores=N_CORES):
    """v3: concat scheme (as v1) with stationary reuse.

    All weights for one population stay SBUF-resident (8MB fp8); the matmul
    loop is m -> half -> kd -> nb so one LDWEIGHTS serves 4 matmuls (one per
    o-block), cutting LDW traffic 4x and keeping the PE stream dense. PSUM
    holds 4 accumulating banks (one per o-block) per m-subtile.
    """
    kt = i_dim // PART
    nb = o_dim // 512
    mb = b // PART
    DR = mybir.MatmulPerfMode.DoubleRow
    nk = kt // 2

    nc = bacc.Bacc("TRN2", target_bir_lowering=False, debug=False,
                   num_devices=n_cores)

    xt_d = nc.dram_tensor("xt", [ppc, PART, kt, b], FP8, kind="ExternalInput")
    w0_d = nc.dram_tensor("w0", [ppc, nb, PART, kt, 512], FP8, kind="ExternalInput")
    w1_d = nc.dram_tensor("w1", [ppc, nb, PART, kt, 512], FP8, kind="ExternalInput")
    out_d = nc.dram_tensor("out", [ppc, b, o_dim], F32, kind="ExternalOutput")

    with tile.TileContext(nc) as tc:
        with (
            tc.tile_pool(name="xpool", bufs=2) as xpool,
            tc.tile_pool(name="wpool", bufs=2 * nb * 2) as wpool,
            tc.tile_pool(name="opool", bufs=6) as opool,
            tc.tile_pool(name="pspool", bufs=8, space="PSUM") as pspool,
        ):
            for pop in range(ppc):
                xt = xpool.tile([PART, kt, b], FP8, tag="xt")
                nxt = xpool.tile([PART, kt, b], FP8, tag="nxt")
                nc.gpsimd.dma_start(out=xt[:], in_=xt_d.ap()[pop])
                nc.vector.tensor_scalar(
                    nxt[:], xt[:], -1.0, 1.0,
                    mybir.AluOpType.mult, mybir.AluOpType.add,
                )
                # all weights for this pop, k-chunked so matmuls start early;
                # w0 on the sync HWDGE ring, w1 on the scalar HWDGE ring
                w0t = [wpool.tile([PART, kt, 512], FP8, tag="w",
                                  name=f"w0t_{pop}_{i}") for i in range(nb)]
                w1t = [wpool.tile([PART, kt, 512], FP8, tag="w",
                                  name=f"w1t_{pop}_{i}") for i in range(nb)]
                for ch in range(0, kt, 4):
                    for nbi in range(nb):
                        nc.sync.dma_start(
                            out=w0t[nbi][:, ch:ch + 4, :],
                            in_=w0_d.ap()[pop, nbi, :, ch:ch + 4, :])
                        nc.scalar.dma_start(
                            out=w1t[nbi][:, ch:ch + 4, :],
                            in_=w1_d.ap()[pop, nbi, :, ch:ch + 4, :])
                for m in range(mb):
                    msl = slice(m * PART, (m + 1) * PART)
                    pss = [pspool.tile([PART, 512], F32, tag="ps",
                                       name=f"ps_{pop}_{m}_{i}") for i in range(nb)]
                    for half, (xsrc, wt) in enumerate(((xt, w0t), (nxt, w1t))):
                        for kd in range(nk):
                            ksl = slice(2 * kd, 2 * kd + 2)
                            for nbi in range(nb):
                                nc.tensor.matmul(
                                    pss[nbi][:], lhsT=xsrc[:, ksl, msl],
                                    rhs=wt[nbi][:, ksl, :],
                                    start=(half == 0 and kd == 0),
                                    stop=(half == 1 and kd == nk - 1),
                                    perf_mode=DR,
                                )
                    for nbi in range(nb):
                        ot = opool.tile([PART, 512], F32)
                        nc.vector.tensor_copy(ot[:], pss[nbi][:])
                        nc.gpsimd.dma_start(
                            out=out_d.ap()[pop, msl, nbi * 512:(nbi + 1) * 512],
                            in_=ot[:],
                        )
    nc.compile()
    return nc


def build_nc_v4(ppc=PPC, b=B, i_dim=I, o_dim=O, n_cores=N_CORES):
    """v4: out = x@(w0-w1) + colsum(w1), wd built by DVE+gpsimd tensor_tensor.

    Halves the PE matmul stream vs the concat scheme (K=2048 instead of 4096).
    Per o-block: load w0/w1, bias = colsum(w1) via an all-ones DR matmul,
    wd = w0-w1 with the k-subtiles split between vector (11) and gpsimd (5)
    engines, main matmuls accumulate x@wd, and the DVE evacuation adds bias
    (tensor_tensor add against a bias tile copied from the bias PSUM bank).
    """
    kt = i_dim // PART
    nb = o_dim // 512
    mb = b // PART
    DR = mybir.MatmulPerfMode.DoubleRow
    nk = kt // 2
    # all subtract work on DVE: offloading 2 k-subtiles to gpsimd measured
    # 128.6us vs 128.0us all-DVE — the DVE's 23us of idle means it is not
    # strictly binding, and the gpsimd offload does not pay
    kdve = kt

    nc = bacc.Bacc("TRN2", target_bir_lowering=False, debug=False,
                   num_devices=n_cores)

    xt_d = nc.dram_tensor("xt", [ppc, PART, kt, b], FP8, kind="ExternalInput")
    w0_d = nc.dram_tensor("w0", [ppc, nb, PART, kt, 512], FP8, kind="ExternalInput")
    w1_d = nc.dram_tensor("w1", [ppc, nb, PART, kt, 512], FP8, kind="ExternalInput")
    out_d = nc.dram_tensor("out", [ppc, b, o_dim], F32, kind="ExternalOutput")

    with tile.TileContext(nc) as tc:
        with (
            tc.tile_pool(name="const", bufs=1) as const,
            tc.tile_pool(name="xpool", bufs=2) as xpool,
            tc.tile_pool(name="wsrc", bufs=6) as wsrc,
            tc.tile_pool(name="wdpool", bufs=4) as wdpool,
            tc.tile_pool(name="bpool", bufs=3) as bpool,
            tc.tile_pool(name="opool", bufs=4) as opool,
            tc.tile_pool(name="pspool", bufs=4, space="PSUM") as pspool,
            tc.tile_pool(name="psbias", bufs=2, space="PSUM") as psbias,
        ):
            ones = const.tile([PART, 2, PART], FP8)
            nc.vector.memset(ones[:], 1.0)
            xts = {}
            state = {}
            blocks = [(pop, nbi) for pop in range(ppc) for nbi in range(nb)]

            def prepare(pop, nbi):
                if nbi == 0:
                    xt = xpool.tile([PART, kt, b], FP8, tag="xt",
                                    name=f"xt_{pop}")
                    xch = min(4, kt)
                    for ch in range(0, kt, xch):
                        nc.scalar.dma_start(
                            out=xt[:, ch:ch + xch, :],
                            in_=xt_d.ap()[pop, :, ch:ch + xch, :])
                    xts[pop] = xt
                w0t = wsrc.tile([PART, kt, 512], FP8, tag="ws",
                                name=f"w0t_{pop}_{nbi}")
                w1t = wsrc.tile([PART, kt, 512], FP8, tag="ws",
                                name=f"w1t_{pop}_{nbi}")
                wch = 2 if (pop == 0 and nbi == 0) else 4
                for ch in range(0, kt, wch):
                    nc.sync.dma_start(
                        out=w1t[:, ch:ch + wch, :],
                        in_=w1_d.ap()[pop, nbi, :, ch:ch + wch, :])
                    nc.scalar.dma_start(
                        out=w0t[:, ch:ch + wch, :],
                        in_=w0_d.ap()[pop, nbi, :, ch:ch + wch, :])
                # bias = colsum(w1) (all rows of psb identical)
                psb = psbias.tile([PART, 512], F32, tag="psb")
                for kd in range(nk):
                    ksl = slice(2 * kd, 2 * kd + 2)
                    nc.tensor.matmul(
                        psb[:], lhsT=ones[:], rhs=w1t[:, ksl, :],
                        start=(kd == 0), stop=(kd == nk - 1), perf_mode=DR)
                bias_sb = bpool.tile([PART, 512], F32, tag="bias")
                nc.vector.tensor_copy(bias_sb[:], psb[:])
                # wd = w0 - w1 on DVE in fine k-chunks; emitted one block
                # AHEAD of the consuming matmuls (software pipeline) so these
                # sit before the previous block's evacuations in the DVE FIFO
                wd = wdpool.tile([PART, kt, 512], FP8, tag="wd")
                sch = max(1, kt // 8)
                for ch in range(0, kdve, sch):
                    nc.vector.tensor_tensor(
                        wd[:, ch:ch + sch, :], w0t[:, ch:ch + sch, :],
                        w1t[:, ch:ch + sch, :], mybir.AluOpType.subtract)
                if kdve < kt:
                    nc.gpsimd.tensor_tensor(
                        wd[:, kdve:, :], w0t[:, kdve:, :], w1t[:, kdve:, :],
                        mybir.AluOpType.subtract)
                state[(pop, nbi)] = (wd, bias_sb)

            def main(pop, nbi):
                wd, bias_sb = state.pop((pop, nbi))
                xt = xts[pop]
                for m in range(mb):
                    ps = pspool.tile([PART, 512], F32, tag="ps",
                                     name=f"ps_{pop}_{nbi}_{m}")
                    msl = slice(m * PART, (m + 1) * PART)
                    for kd in range(nk):
                        ksl = slice(2 * kd, 2 * kd + 2)
                        nc.tensor.matmul(
                            ps[:], lhsT=xt[:, ksl, msl], rhs=wd[:, ksl, :],
                            start=(kd == 0), stop=(kd == nk - 1), perf_mode=DR)
                    ot = opool.tile([PART, 512], F32, tag="ot",
                                    name=f"ot_{pop}_{nbi}_{m}")
                    nc.vector.tensor_tensor(
                        ot[:], ps[:], bias_sb[:], mybir.AluOpType.add)
                    nc.gpsimd.dma_start(
                        out=out_d.ap()[pop, msl, nbi * 512:(nbi + 1) * 512],
                        in_=ot[:])

            for i in range(len(blocks) + 1):
                if i < len(blocks):
                    prepare(*blocks[i])
                if i > 0:
                    main(*blocks[i - 1])
    nc.compile()
    return nc


def build_nc_v2(ppc=PPC, b=B, i_dim=I, o_dim=O, n_cores=N_CORES):
    """v2: algebraic rewrite out = x@(w0-w1) + colsum(w1).

    The w1 input tensor holds -w1 (sign applied during the host fp8 cast;
    walrus rejects cce_op=subtract but accepts add):
    - wd = w0 + (-w1) computed by the gpsimd DMA inline ALU (accum_op=add)
      while loading w0 — zero compute-engine cost.
    - colsum(-w1) = -bias via an all-ones stationary matmul against the tile
      while it still holds -w1, once per o-block.
    - main pass: psum = x @ wd, half the PE work of v1; evacuated as
      psum - (-bias) with a DVE tensor_tensor subtract.
    All values stay exact: x in {0,1}, wd in {-1,0,1} (fp8 exact), bias and
    accumulation in f32 (integers < 2^24).
    """
    kt = i_dim // PART
    nb = o_dim // 512
    mb = b // PART
    DR = mybir.MatmulPerfMode.DoubleRow
    nk = kt // 2

    nc = bacc.Bacc("TRN2", target_bir_lowering=False, debug=False,
                   num_devices=n_cores)

    xt_d = nc.dram_tensor("xt", [ppc, PART, kt, b], FP8, kind="ExternalInput")
    w0_d = nc.dram_tensor("w0", [ppc, nb, PART, kt, 512], FP8, kind="ExternalInput")
    w1_d = nc.dram_tensor("w1", [ppc, nb, PART, kt, 512], FP8, kind="ExternalInput")
    out_d = nc.dram_tensor("out", [ppc, b, o_dim], F32, kind="ExternalOutput")

    with tile.TileContext(nc) as tc:
        with (
            tc.tile_pool(name="const", bufs=1) as const,
            tc.tile_pool(name="xpool", bufs=2) as xpool,
            tc.tile_pool(name="wpool", bufs=4) as wpool,
            tc.tile_pool(name="bpool", bufs=2) as bpool,
            tc.tile_pool(name="opool", bufs=4) as opool,
            tc.tile_pool(name="pspool", bufs=4, space="PSUM") as pspool,
            tc.tile_pool(name="psbias", bufs=2, space="PSUM") as psbias,
        ):
            ones = const.tile([PART, 2, PART], FP8)
            nc.vector.memset(ones[:], 1.0)
            for pop in range(ppc):
                xt = xpool.tile([PART, kt, b], FP8, tag="xt")
                nc.scalar.dma_start(out=xt[:], in_=xt_d.ap()[pop])
                for nbi in range(nb):
                    # 544-wide rows (512 data + 32 pad): keeps every SBUF write
                    # run at 512B so the accum DMA's RMW ucode accepts it (runs
                    # >512B crash the exec unit), and stops the AP optimizer
                    # from merging rows into one big run.
                    wdp = wpool.tile([PART, kt, 544], FP8, tag="w")
                    wd = wdp[:, :, :512]
                    # 1) load -w1 (sync HWDGE ring)
                    wch = min(8, kt)
                    for ch in range(0, kt, wch):
                        nc.sync.dma_start(
                            out=wd[:, ch:ch + wch, :],
                            in_=w1_d.ap()[pop, nbi, :, ch:ch + wch, :])
                    # 2) -bias = colsum(-w1) while the tile still holds -w1
                    psb = psbias.tile([PART, 512], F32)
                    for kd in range(nk):
                        ksl = slice(2 * kd, 2 * kd + 2)
                        nc.tensor.matmul(
                            psb[:], lhsT=ones[:], rhs=wd[:, ksl, :],
                            start=(kd == 0), stop=(kd == nk - 1), perf_mode=DR)
                    bias_sb = bpool.tile([PART, 512], F32, tag="bias")
                    nc.vector.tensor_copy(bias_sb[:], psb[:])
                    # 3) wd = w0 + (-w1) via DMA inline ALU (op(in,out) = in+out)
                    nc.gpsimd.dma_start(out=wd[:], in_=w0_d.ap()[pop, nbi],
                                        accum_op=mybir.AluOpType.add)
                    # 4) main pass: psum = x @ wd, evac with bias add
                    for m in range(mb):
                        ps = pspool.tile([PART, 512], F32)
                        msl = slice(m * PART, (m + 1) * PART)
                        for kd in range(nk):
                            ksl = slice(2 * kd, 2 * kd + 2)
                            nc.tensor.matmul(
                                ps[:], lhsT=xt[:, ksl, msl], rhs=wd[:, ksl, :],
                                start=(kd == 0), stop=(kd == nk - 1), perf_mode=DR)
                        ot = opool.tile([PART, 512], F32)
                        # out = psum - (-bias)
                        nc.vector.tensor_tensor(
                            ot[:], ps[:], bias_sb[:], mybir.AluOpType.subtract)
                        nc.scalar.dma_start(
                            out=out_d.ap()[pop, msl, nbi * 512:(nbi + 1) * 512],
                            in_=ot[:])
    nc.compile()
    return nc


def prep_core_inputs(x, w, core, ppc=PPC, negate_w1=False):
    """Layout-only host prep for one core: slice pops, transpose x, tile, cast.
    With negate_w1, the fp8 cast of w1 carries a sign flip (v2 sends -w1 so the
    device can form w0-w1 with the DMA ALU's accum add)."""
    p0 = core * ppc
    b, i_dim = x.shape[1], x.shape[2]
    o_dim = w.shape[4]
    kt = i_dim // PART
    nb = o_dim // 512
    xs = x[p0:p0 + ppc]                       # [ppc, B, I]
    # xT partition-tiled: [ppc, 128, kt, B];  xt[p, kp, kti, b] = x[p, b, kti*128+kp]
    xt = np.ascontiguousarray(
        xs.reshape(ppc, b, kt, PART).transpose(0, 3, 2, 1)
    ).astype(NP_FP8)
    ws = w[:, p0:p0 + ppc, 0]                 # [2, ppc, I, O]
    # [2, ppc, nb, 128, kt, 512]; wt[j,p,nbi,kp,kti,no] = w[j,p,kti*128+kp, nbi*512+no]
    wt = np.ascontiguousarray(
        ws.reshape(2, ppc, kt, PART, nb, 512).transpose(0, 1, 4, 3, 2, 5)
    )
    w0 = wt[0].astype(NP_FP8)
    w1 = (-wt[1]).astype(NP_FP8) if negate_w1 else wt[1].astype(NP_FP8)
    return {"xt": xt, "w0": w0, "w1": w1}


_NC_CACHE = {}

# which builder kernel() uses: 1 = concat (x@w0 + notx@w1), 2 = DMA-subtract trick
K_VERSION = int(os.environ.get("EVO_KERNEL_VERSION", "4"))


def _get_nc():
    if "nc" not in _NC_CACHE:
        builder = {1: build_nc, 2: build_nc_v2, 3: build_nc_v3,
                   4: build_nc_v4}[K_VERSION]
        _NC_CACHE["nc"] = builder()
    return _NC_CACHE["nc"]


def kernel(x, w):
    x = np.asarray(x)
    w = np.asarray(w)
    nc = _get_nc()
    in_maps = [prep_core_inputs(x, w, c, negate_w1=(K_VERSION == 2))
               for c in range(N_CORES)]
    res = run_bass_kernel_spmd(nc, in_maps, list(range(N_CORES)))
    out = np.concatenate([res.results[c]["out"] for c in range(N_CORES)], axis=0)
    return np.ascontiguousarray(out.astype(np.float32))



# revision 6
# speedup vs baseline: 1.0863x; 1.0863x over previous
"""Bass/Trainium2 kernel for nn_EvoBinarizedLayer.

Reference computation (P=16 populations, B=512, I=O=2048, all values 0/1):
    out[p,b,o] = sum_i x[p,b,i]*w0[p,i,o] + (1-x[p,b,i])*w1[p,i,o]

Strategy:
  - Shard population dim P across 8 cores (2 pops/core), embarrassingly parallel.
  - Cast x/w to fp8e4m3 on host (0/1 values are exact); compute notx = 1-x on
    device (ACT/DVE); accumulate x@w0 + notx@w1 into the same PSUM bank via a
    single K=4096 "concat" contraction -> one accumulation group, no bias pass.
  - fp8 DoubleRow matmuls (K=256 per MM) for 2x PE throughput.
  - PSUM f32 accumulation of 0/1 products is exact (max 4096 < 2^24), so the
    result is bit-exact vs the f32 reference.

Host-side work is layout only: slicing, transpose, dtype cast, and the final
gather. All arithmetic (notx, matmuls) happens on device.
"""

import os

import numpy as np
import ml_dtypes

from concourse import bacc, tile, mybir
from concourse.bass_utils import run_bass_kernel_spmd

P_TOT, B, I, O = 16, 512, 2048, 2048
N_CORES = 8
PPC = P_TOT // N_CORES  # pops per core = 2
PART = 128

FP8 = mybir.dt.float8e4
F16 = mybir.dt.float16
F32 = mybir.dt.float32
NP_FP8 = ml_dtypes.float8_e4m3


def build_nc(ppc=PPC, b=B, i_dim=I, o_dim=O, n_cores=N_CORES, use_dr=True):
    """Build + compile the per-core Bass program (SPMD: same program, 8 cores)."""
    kt = i_dim // PART          # k-subtiles per weight tensor (16)
    nb = o_dim // 512           # o-blocks (4)
    mb = b // PART              # b-subtiles (4)
    DR = mybir.MatmulPerfMode.DoubleRow if use_dr else None
    kstep = 2 if use_dr else 1

    nc = bacc.Bacc("TRN2", target_bir_lowering=False, debug=False,
                   num_devices=n_cores)

    xt_d = nc.dram_tensor("xt", [ppc, PART, kt, b], FP8, kind="ExternalInput")
    w0_d = nc.dram_tensor("w0", [ppc, nb, PART, kt, 512], FP8, kind="ExternalInput")
    w1_d = nc.dram_tensor("w1", [ppc, nb, PART, kt, 512], FP8, kind="ExternalInput")
    out_d = nc.dram_tensor("out", [ppc, b, o_dim], F32, kind="ExternalOutput")

    with tile.TileContext(nc) as tc:
        with (
            tc.tile_pool(name="warm", bufs=1) as warm,
            tc.tile_pool(name="xpool", bufs=2) as xpool,
            tc.tile_pool(name="wpool", bufs=8) as wpool,
            tc.tile_pool(name="opool", bufs=4) as opool,
            tc.tile_pool(name="pspool", bufs=4, space="PSUM") as pspool,
            tc.tile_pool(name="warmps", bufs=1, space="PSUM") as warmps,
        ):
            for pop in range(ppc):
                xt = xpool.tile([PART, kt, b], FP8, tag="xt")
                nxt = xpool.tile([PART, kt, b], FP8, tag="nxt")
                # x chunked on the scalar ring ahead of w1: the first matmul
                # needs only xt[:, 0:2, :], so a 256KB first chunk unblocks
                # the first LDWEIGHTS ~10us sooner than one 1MB transfer.
                xch = min(4, kt)
                for ch in range(0, kt, xch):
                    nc.scalar.dma_start(out=xt[:, ch:ch + xch, :],
                                        in_=xt_d.ap()[pop, :, ch:ch + xch, :])
                    # notx = 1 - x  ==  (x * -1) + 1, per chunk
                    nc.vector.tensor_scalar(
                        nxt[:, ch:ch + xch, :], xt[:, ch:ch + xch, :], -1.0, 1.0,
                        mybir.AluOpType.mult, mybir.AluOpType.add,
                    )
                for nbi in range(nb):
                    w0t = wpool.tile([PART, kt, 512], FP8, tag="w")
                    w1t = wpool.tile([PART, kt, 512], FP8, tag="w")
                    # w0 loads on the sync HWDGE ring, w1 on the scalar HWDGE
                    # ring (output stores go via gpsimd/SWDGE) so stores never
                    # block weight prefetch in a shared FIFO. Chunked k-wise so
                    # the first matmuls start before the whole block lands; the
                    # very first block uses finer chunks to cut the startup
                    # bubble before the first LDWEIGHTS.
                    wch = 2 if (pop == 0 and nbi == 0) else 4
                    for ch in range(0, kt, wch):
                        nc.sync.dma_start(
                            out=w0t[:, ch:ch + wch, :],
                            in_=w0_d.ap()[pop, nbi, :, ch:ch + wch, :])
                        nc.scalar.dma_start(
                            out=w1t[:, ch:ch + wch, :],
                            in_=w1_d.ap()[pop, nbi, :, ch:ch + wch, :])
                    for m in range(mb):
                        ps = pspool.tile([PART, 512], F32)
                        msl = slice(m * PART, (m + 1) * PART)
                        nk = kt // kstep
                        for kd in range(nk):
                            ksl = slice(kd * kstep, (kd + 1) * kstep)
                            nc.tensor.matmul(
                                ps[:], lhsT=xt[:, ksl, msl], rhs=w0t[:, ksl, :],
                                start=(kd == 0), stop=False, perf_mode=DR,
                            )
                        for kd in range(nk):
                            ksl = slice(kd * kstep, (kd + 1) * kstep)
                            nc.tensor.matmul(
                                ps[:], lhsT=nxt[:, ksl, msl], rhs=w1t[:, ksl, :],
                                start=False, stop=(kd == nk - 1), perf_mode=DR,
                            )
                        ot = opool.tile([PART, 512], F32)
                        nc.vector.tensor_copy(ot[:], ps[:])
                        nc.gpsimd.dma_start(
                            out=out_d.ap()[pop, msl, nbi * 512:(nbi + 1) * 512],
                            in_=ot[:],
                        )
    nc.compile()
    return nc


def build_nc_v3(ppc=PPC, b=B, i_dim=I, o_dim=O, n_cores=N_CORES):
    """v3: concat scheme (as v1) with stationary reuse.

    All weights for one population stay SBUF-resident (8MB fp8); the matmul
    loop is m -> half -> kd -> nb so one LDWEIGHTS serves 4 matmuls (one per
    o-block), cutting LDW traffic 4x and keeping the PE stream dense. PSUM
    holds 4 accumulating banks (one per o-block) per m-subtile.
    """
    kt = i_dim // PART
    nb = o_dim // 512
    mb = b // PART
    DR = mybir.MatmulPerfMode.DoubleRow
    nk = kt // 2

    nc = bacc.Bacc("TRN2", target_bir_lowering=False, debug=False,
                   num_devices=n_cores)

    xt_d = nc.dram_tensor("xt", [ppc, PART, kt, b], FP8, kind="ExternalInput")
    w0_d = nc.dram_tensor("w0", [ppc, nb, PART, kt, 512], FP8, kind="ExternalInput")
    w1_d = nc.dram_tensor("w1", [ppc, nb, PART, kt, 512], FP8, kind="ExternalInput")
    out_d = nc.dram_tensor("out", [ppc, b, o_dim], F32, kind="ExternalOutput")

    with tile.TileContext(nc) as tc:
        with (
            tc.tile_pool(name="xpool", bufs=2) as xpool,
            tc.tile_pool(name="wpool", bufs=2 * nb * 2) as wpool,
            tc.tile_pool(name="opool", bufs=6) as opool,
            tc.tile_pool(name="pspool", bufs=8, space="PSUM") as pspool,
        ):
            for pop in range(ppc):
                xt = xpool.tile([PART, kt, b], FP8, tag="xt")
                nxt = xpool.tile([PART, kt, b], FP8, tag="nxt")
                nc.gpsimd.dma_start(out=xt[:], in_=xt_d.ap()[pop])
                nc.vector.tensor_scalar(
                    nxt[:], xt[:], -1.0, 1.0,
                    mybir.AluOpType.mult, mybir.AluOpType.add,
                )
                # all weights for this pop, k-chunked so matmuls start early;
                # w0 on the sync HWDGE ring, w1 on the scalar HWDGE ring
                w0t = [wpool.tile([PART, kt, 512], FP8, tag="w",
                                  name=f"w0t_{pop}_{i}") for i in range(nb)]
                w1t = [wpool.tile([PART, kt, 512], FP8, tag="w",
                                  name=f"w1t_{pop}_{i}") for i in range(nb)]
                for ch in range(0, kt, 4):
                    for nbi in range(nb):
                        nc.sync.dma_start(
                            out=w0t[nbi][:, ch:ch + 4, :],
                            in_=w0_d.ap()[pop, nbi, :, ch:ch + 4, :])
                        nc.scalar.dma_start(
                            out=w1t[nbi][:, ch:ch + 4, :],
                            in_=w1_d.ap()[pop, nbi, :, ch:ch + 4, :])
                for m in range(mb):
                    msl = slice(m * PART, (m + 1) * PART)
                    pss = [pspool.tile([PART, 512], F32, tag="ps",
                                       name=f"ps_{pop}_{m}_{i}") for i in range(nb)]
                    for half, (xsrc, wt) in enumerate(((xt, w0t), (nxt, w1t))):
                        for kd in range(nk):
                            ksl = slice(2 * kd, 2 * kd + 2)
                            for nbi in range(nb):
                                nc.tensor.matmul(
                                    pss[nbi][:], lhsT=xsrc[:, ksl, msl],
                                    rhs=wt[nbi][:, ksl, :],
                                    start=(half == 0 and kd == 0),
                                    stop=(half == 1 and kd == nk - 1),
                                    perf_mode=DR,
                                )
                    for nbi in range(nb):
                        ot = opool.tile([PART, 512], F32)
                        nc.vector.tensor_copy(ot[:], pss[nbi][:])
                        nc.gpsimd.dma_start(
                            out=out_d.ap()[pop, msl, nbi * 512:(nbi + 1) * 512],
                            in_=ot[:],
                        )
    nc.compile()
    return nc


def build_nc_v4(ppc=PPC, b=B, i_dim=I, o_dim=O, n_cores=N_CORES):
    """v4: out = x@(w0-w1) + colsum(w1), wd built by DVE+gpsimd tensor_tensor.

    Halves the PE matmul stream vs the concat scheme (K=2048 instead of 4096).
    Per o-block: load w0/w1, bias = colsum(w1) via an all-ones DR matmul,
    wd = w0-w1 with the k-subtiles split between vector (11) and gpsimd (5)
    engines, main matmuls accumulate x@wd, and the DVE evacuation adds bias
    (tensor_tensor add against a bias tile copied from the bias PSUM bank).
    """
    kt = i_dim // PART
    nb = o_dim // 512
    mb = b // PART
    DR = mybir.MatmulPerfMode.DoubleRow
    nk = kt // 2
    # all subtract work on DVE: offloading 2 k-subtiles to gpsimd measured
    # 128.6us vs 128.0us all-DVE — the DVE's 23us of idle means it is not
    # strictly binding, and the gpsimd offload does not pay
    kdve = kt

    nc = bacc.Bacc("TRN2", target_bir_lowering=False, debug=False,
                   num_devices=n_cores)

    xt_d = nc.dram_tensor("xt", [ppc, PART, kt, b], FP8, kind="ExternalInput")
    w0_d = nc.dram_tensor("w0", [ppc, nb, PART, kt, 512], FP8, kind="ExternalInput")
    w1_d = nc.dram_tensor("w1", [ppc, nb, PART, kt, 512], FP8, kind="ExternalInput")
    out_d = nc.dram_tensor("out", [ppc, b, o_dim], F32, kind="ExternalOutput")

    with tile.TileContext(nc) as tc:
        with (
            tc.tile_pool(name="const", bufs=1) as const,
            tc.tile_pool(name="xpool", bufs=2) as xpool,
            tc.tile_pool(name="wsrc", bufs=6) as wsrc,
            tc.tile_pool(name="wdpool", bufs=4) as wdpool,
            tc.tile_pool(name="bpool", bufs=3) as bpool,
            tc.tile_pool(name="opool", bufs=4) as opool,
            tc.tile_pool(name="pspool", bufs=4, space="PSUM") as pspool,
            tc.tile_pool(name="psbias", bufs=2, space="PSUM") as psbias,
        ):
            ones = const.tile([PART, 2, PART], FP8)
            nc.vector.memset(ones[:], 1.0)
            xts = {}
            state = {}
            blocks = [(pop, nbi) for pop in range(ppc) for nbi in range(nb)]

            def prepare(pop, nbi):
                if nbi == 0:
                    xt = xpool.tile([PART, kt, b], FP8, tag="xt",
                                    name=f"xt_{pop}")
                    xch = min(4, kt)
                    for ch in range(0, kt, xch):
                        nc.scalar.dma_start(
                            out=xt[:, ch:ch + xch, :],
                            in_=xt_d.ap()[pop, :, ch:ch + xch, :])
                    xts[pop] = xt
                w0t = wsrc.tile([PART, kt, 512], FP8, tag="ws",
                                name=f"w0t_{pop}_{nbi}")
                w1t = wsrc.tile([PART, kt, 512], FP8, tag="ws",
                                name=f"w1t_{pop}_{nbi}")
                wch = 2 if (pop == 0 and nbi == 0) else 4
                for ch in range(0, kt, wch):
                    nc.sync.dma_start(
                        out=w1t[:, ch:ch + wch, :],
                        in_=w1_d.ap()[pop, nbi, :, ch:ch + wch, :])
                    nc.scalar.dma_start(
                        out=w0t[:, ch:ch + wch, :],
                        in_=w0_d.ap()[pop, nbi, :, ch:ch + wch, :])
                # bias = colsum(w1) (all rows of psb identical)
                psb = psbias.tile([PART, 512], F32, tag="psb")
                for kd in range(nk):
                    ksl = slice(2 * kd, 2 * kd + 2)
                    nc.tensor.matmul(
                        psb[:], lhsT=ones[:], rhs=w1t[:, ksl, :],
                        start=(kd == 0), stop=(kd == nk - 1), perf_mode=DR)
                bias_sb = bpool.tile([PART, 512], F32, tag="bias")
                nc.vector.tensor_copy(bias_sb[:], psb[:])
                # wd = w0 - w1 on DVE in fine k-chunks; emitted one block
                # AHEAD of the consuming matmuls (software pipeline) so these
                # sit before the previous block's evacuations in the DVE FIFO
                wd = wdpool.tile([PART, kt, 512], FP8, tag="wd")
                sch = max(1, kt // 8)
                for ch in range(0, kdve, sch):
                    nc.vector.tensor_tensor(
                        wd[:, ch:ch + sch, :], w0t[:, ch:ch + sch, :],
                        w1t[:, ch:ch + sch, :], mybir.AluOpType.subtract)
                if kdve < kt:
                    nc.gpsimd.tensor_tensor(
                        wd[:, kdve:, :], w0t[:, kdve:, :], w1t[:, kdve:, :],
                        mybir.AluOpType.subtract)
                state[(pop, nbi)] = (wd, bias_sb)

            def main(pop, nbi):
                wd, bias_sb = state.pop((pop, nbi))
                xt = xts[pop]
                for m in range(mb):
                    ps = pspool.tile([PART, 512], F32, tag="ps",
                                     name=f"ps_{pop}_{nbi}_{m}")
                    msl = slice(m * PART, (m + 1) * PART)
                    for kd in range(nk):
                        ksl = slice(2 * kd, 2 * kd + 2)
                        nc.tensor.matmul(
                            ps[:], lhsT=xt[:, ksl, msl], rhs=wd[:, ksl, :],
                            start=(kd == 0), stop=(kd == nk - 1), perf_mode=DR)
                    ot = opool.tile([PART, 512], F32, tag="ot",
                                    name=f"ot_{pop}_{nbi}_{m}")
                    nc.vector.tensor_tensor(
                        ot[:], ps[:], bias_sb[:], mybir.AluOpType.add)
                    nc.gpsimd.dma_start(
                        out=out_d.ap()[pop, msl, nbi * 512:(nbi + 1) * 512],
                        in_=ot[:])

            for i in range(len(blocks) + 1):
                if i < len(blocks):
                    prepare(*blocks[i])
                if i > 0:
                    main(*blocks[i - 1])
    nc.compile()
    return nc


def build_nc_v5(ppc=PPC, b=B, i_dim=I, o_dim=O, n_cores=N_CORES, look=2):
    """v5: v2's DMA-accum wd trick + v4's software pipelining + fp16 output.

    Per (pop, nbi) block:
      prepare: load -w1 (sync HWDGE) into a 544-strided tile (512B SBUF write
               runs keep the accum ucode legal); -bias = colsum(-w1) via ones
               DR matmuls (PE); bias copied psum->SBUF on ACT; wd = w0 + (-w1)
               via the SWDGE inline-ALU accum DMA — zero compute-engine cost.
      main:    4 m-subtiles x 8 DR matmuls accumulate x@wd in PSUM; DVE
               evacuates psum - (-bias) straight to fp16; scalar-ring DMA
               stores fp16 to DRAM (half the store traffic; all values are
               integers <= 2048, fp16-exact).
    prepare runs `look` blocks ahead of main so the serial w1-load -> bias-MM
    -> accum-DMA chain hides under earlier blocks' main matmuls and the PE
    never idles long enough to re-throttle (HAM).
    """
    kt = i_dim // PART          # 16
    nb = o_dim // 512           # 4
    mb = b // PART              # 4
    DR = mybir.MatmulPerfMode.DoubleRow
    nk = kt // 2                # 8

    nc = bacc.Bacc("TRN2", target_bir_lowering=False, debug=False,
                   num_devices=n_cores)

    xt_d = nc.dram_tensor("xt", [ppc, PART, kt, b], FP8, kind="ExternalInput")
    w0_d = nc.dram_tensor("w0", [ppc, nb, PART, kt, 512], FP8, kind="ExternalInput")
    w1_d = nc.dram_tensor("w1", [ppc, nb, PART, kt, 512], FP8, kind="ExternalInput")
    out_d = nc.dram_tensor("out", [ppc, b, o_dim], F16, kind="ExternalOutput")

    with tile.TileContext(nc) as tc:
        with (
            tc.tile_pool(name="const", bufs=1) as const,
            tc.tile_pool(name="xpool", bufs=2) as xpool,
            tc.tile_pool(name="wpool", bufs=look + 2) as wpool,
            tc.tile_pool(name="bpool", bufs=look + 2) as bpool,
            tc.tile_pool(name="opool", bufs=6) as opool,
            tc.tile_pool(name="pspool", bufs=4, space="PSUM") as pspool,
            tc.tile_pool(name="psbias", bufs=3, space="PSUM") as psbias,
        ):
            ones = const.tile([PART, 2, PART], FP8)
            nc.vector.memset(ones[:], 1.0)
            xts = {}
            state = {}
            blocks = [(pop, nbi) for pop in range(ppc) for nbi in range(nb)]

            def prepare(i):
                pop, nbi = blocks[i]
                if nbi == 0:
                    xt = xpool.tile([PART, kt, b], FP8, tag="xt",
                                    name=f"xt_{pop}")
                    xch = 4 if pop == 0 else kt
                    for ch in range(0, kt, xch):
                        nc.scalar.dma_start(out=xt[:, ch:ch + xch, :],
                                            in_=xt_d.ap()[pop, :, ch:ch + xch, :])
                    xts[pop] = xt
                wdp = wpool.tile([PART, kt, 544], FP8, tag="w", name=f"wd_{i}")
                wd = wdp[:, :, :512]
                wch = 4 if i == 0 else 8
                for ch in range(0, kt, wch):
                    nc.sync.dma_start(out=wd[:, ch:ch + wch, :],
                                      in_=w1_d.ap()[pop, nbi, :, ch:ch + wch, :])
                psb = psbias.tile([PART, 512], F32, tag="psb", name=f"psb_{i}")
                for kd in range(nk):
                    ksl = slice(2 * kd, 2 * kd + 2)
                    nc.tensor.matmul(psb[:], lhsT=ones[:], rhs=wd[:, ksl, :],
                                     start=(kd == 0), stop=(kd == nk - 1),
                                     perf_mode=DR)
                bias_sb = bpool.tile([PART, 512], F32, tag="bias",
                                     name=f"bias_{i}")
                nc.scalar.copy(bias_sb[:], psb[:])
                nc.gpsimd.dma_start(out=wd[:], in_=w0_d.ap()[pop, nbi],
                                    accum_op=mybir.AluOpType.add)
                state[i] = (wd, bias_sb)

            def main(i):
                pop, nbi = blocks[i]
                wd, bias_sb = state.pop(i)
                xt = xts[pop]
                for m in range(mb):
                    ps = pspool.tile([PART, 512], F32, tag="ps",
                                     name=f"ps_{i}_{m}")
                    msl = slice(m * PART, (m + 1) * PART)
                    for kd in range(nk):
                        ksl = slice(2 * kd, 2 * kd + 2)
                        nc.tensor.matmul(ps[:], lhsT=xt[:, ksl, msl],
                                         rhs=wd[:, ksl, :],
                                         start=(kd == 0), stop=(kd == nk - 1),
                                         perf_mode=DR)
                    ot = opool.tile([PART, 512], F16, tag="ot",
                                    name=f"ot_{i}_{m}")
                    # out = psum - (-bias), evacuated straight to fp16
                    nc.vector.tensor_tensor(ot[:], ps[:], bias_sb[:],
                                            mybir.AluOpType.subtract)
                    nc.scalar.dma_start(
                        out=out_d.ap()[pop, msl, nbi * 512:(nbi + 1) * 512],
                        in_=ot[:])

            n = len(blocks)
            for i in range(n + look):
                if i < n:
                    prepare(i)
                if i >= look:
                    main(i - look)
    nc.compile()
    return nc


def build_nc_v2(ppc=PPC, b=B, i_dim=I, o_dim=O, n_cores=N_CORES):
    """v2: algebraic rewrite out = x@(w0-w1) + colsum(w1).

    The w1 input tensor holds -w1 (sign applied during the host fp8 cast;
    walrus rejects cce_op=subtract but accepts add):
    - wd = w0 + (-w1) computed by the gpsimd DMA inline ALU (accum_op=add)
      while loading w0 — zero compute-engine cost.
    - colsum(-w1) = -bias via an all-ones stationary matmul against the tile
      while it still holds -w1, once per o-block.
    - main pass: psum = x @ wd, half the PE work of v1; evacuated as
      psum - (-bias) with a DVE tensor_tensor subtract.
    All values stay exact: x in {0,1}, wd in {-1,0,1} (fp8 exact), bias and
    accumulation in f32 (integers < 2^24).
    """
    kt = i_dim // PART
    nb = o_dim // 512
    mb = b // PART
    DR = mybir.MatmulPerfMode.DoubleRow
    nk = kt // 2

    nc = bacc.Bacc("TRN2", target_bir_lowering=False, debug=False,
                   num_devices=n_cores)

    xt_d = nc.dram_tensor("xt", [ppc, PART, kt, b], FP8, kind="ExternalInput")
    w0_d = nc.dram_tensor("w0", [ppc, nb, PART, kt, 512], FP8, kind="ExternalInput")
    w1_d = nc.dram_tensor("w1", [ppc, nb, PART, kt, 512], FP8, kind="ExternalInput")
    out_d = nc.dram_tensor("out", [ppc, b, o_dim], F32, kind="ExternalOutput")

    with tile.TileContext(nc) as tc:
        with (
            tc.tile_pool(name="const", bufs=1) as const,
            tc.tile_pool(name="xpool", bufs=2) as xpool,
            tc.tile_pool(name="wpool", bufs=4) as wpool,
            tc.tile_pool(name="bpool", bufs=2) as bpool,
            tc.tile_pool(name="opool", bufs=4) as opool,
            tc.tile_pool(name="pspool", bufs=4, space="PSUM") as pspool,
            tc.tile_pool(name="psbias", bufs=2, space="PSUM") as psbias,
        ):
            ones = const.tile([PART, 2, PART], FP8)
            nc.vector.memset(ones[:], 1.0)
            for pop in range(ppc):
                xt = xpool.tile([PART, kt, b], FP8, tag="xt")
                nc.scalar.dma_start(out=xt[:], in_=xt_d.ap()[pop])
                for nbi in range(nb):
                    # 544-wide rows (512 data + 32 pad): keeps every SBUF write
                    # run at 512B so the accum DMA's RMW ucode accepts it (runs
                    # >512B crash the exec unit), and stops the AP optimizer
                    # from merging rows into one big run.
                    wdp = wpool.tile([PART, kt, 544], FP8, tag="w")
                    wd = wdp[:, :, :512]
                    # 1) load -w1 (sync HWDGE ring)
                    wch = min(8, kt)
                    for ch in range(0, kt, wch):
                        nc.sync.dma_start(
                            out=wd[:, ch:ch + wch, :],
                            in_=w1_d.ap()[pop, nbi, :, ch:ch + wch, :])
                    # 2) -bias = colsum(-w1) while the tile still holds -w1
                    psb = psbias.tile([PART, 512], F32)
                    for kd in range(nk):
                        ksl = slice(2 * kd, 2 * kd + 2)
                        nc.tensor.matmul(
                            psb[:], lhsT=ones[:], rhs=wd[:, ksl, :],
                            start=(kd == 0), stop=(kd == nk - 1), perf_mode=DR)
                    bias_sb = bpool.tile([PART, 512], F32, tag="bias")
                    nc.vector.tensor_copy(bias_sb[:], psb[:])
                    # 3) wd = w0 + (-w1) via DMA inline ALU (op(in,out) = in+out)
                    nc.gpsimd.dma_start(out=wd[:], in_=w0_d.ap()[pop, nbi],
                                        accum_op=mybir.AluOpType.add)
                    # 4) main pass: psum = x @ wd, evac with bias add
                    for m in range(mb):
                        ps = pspool.tile([PART, 512], F32)
                        msl = slice(m * PART, (m + 1) * PART)
                        for kd in range(nk):
                            ksl = slice(2 * kd, 2 * kd + 2)
                            nc.tensor.matmul(
                                ps[:], lhsT=xt[:, ksl, msl], rhs=wd[:, ksl, :],
                                start=(kd == 0), stop=(kd == nk - 1), perf_mode=DR)
                        ot = opool.tile([PART, 512], F32)
                        # out = psum - (-bias)
                        nc.vector.tensor_tensor(
                            ot[:], ps[:], bias_sb[:], mybir.AluOpType.subtract)
                        nc.scalar.dma_start(
                            out=out_d.ap()[pop, msl, nbi * 512:(nbi + 1) * 512],
                            in_=ot[:])
    nc.compile()
    return nc


def prep_core_inputs(x, w, core, ppc=PPC, negate_w1=None):
    """Layout-only host prep for one core: slice pops, transpose x, tile, cast.
    With negate_w1, the fp8 cast of w1 carries a sign flip (v2/v5 send -w1 so
    the device can form w0-w1 with the DMA ALU's accum add). Defaults from
    K_VERSION so test harnesses that call this without the flag stay correct."""
    if negate_w1 is None:
        negate_w1 = K_VERSION in (2, 5)
    p0 = core * ppc
    b, i_dim = x.shape[1], x.shape[2]
    o_dim = w.shape[4]
    kt = i_dim // PART
    nb = o_dim // 512
    xs = x[p0:p0 + ppc]                       # [ppc, B, I]
    # xT partition-tiled: [ppc, 128, kt, B];  xt[p, kp, kti, b] = x[p, b, kti*128+kp]
    xt = np.ascontiguousarray(
        xs.reshape(ppc, b, kt, PART).transpose(0, 3, 2, 1)
    ).astype(NP_FP8)
    ws = w[:, p0:p0 + ppc, 0]                 # [2, ppc, I, O]
    # [2, ppc, nb, 128, kt, 512]; wt[j,p,nbi,kp,kti,no] = w[j,p,kti*128+kp, nbi*512+no]
    wt = np.ascontiguousarray(
        ws.reshape(2, ppc, kt, PART, nb, 512).transpose(0, 1, 4, 3, 2, 5)
    )
    w0 = wt[0].astype(NP_FP8)
    w1 = (-wt[1]).astype(NP_FP8) if negate_w1 else wt[1].astype(NP_FP8)
    return {"xt": xt, "w0": w0, "w1": w1}


_NC_CACHE = {}

# which builder kernel() uses: 1 = concat (x@w0 + notx@w1), 2 = DMA-subtract
# trick, 4 = DVE-subtract pipelined, 5 = DMA-subtract pipelined + fp16 out
K_VERSION = int(os.environ.get("EVO_KERNEL_VERSION", "5"))


def _get_nc():
    if "nc" not in _NC_CACHE:
        builder = {1: build_nc, 2: build_nc_v2, 3: build_nc_v3,
                   4: build_nc_v4, 5: build_nc_v5}[K_VERSION]
        _NC_CACHE["nc"] = builder()
    return _NC_CACHE["nc"]


def kernel(x, w):
    x = np.asarray(x)
    w = np.asarray(w)
    nc = _get_nc()
    in_maps = [prep_core_inputs(x, w, c) for c in range(N_CORES)]
    res = run_bass_kernel_spmd(nc, in_maps, list(range(N_CORES)))
    out = np.concatenate([res.results[c]["out"] for c in range(N_CORES)], axis=0)
    return np.ascontiguousarray(out.astype(np.float32))



# revision 10
# speedup vs baseline: 1.5629x; 1.4387x over previous
"""Bass/Trainium2 kernel for nn_EvoBinarizedLayer.

Reference computation (P=16 populations, B=512, I=O=2048, all values 0/1):
    out[p,b,o] = sum_i x[p,b,i]*w0[p,i,o] + (1-x[p,b,i])*w1[p,i,o]

Strategy:
  - Shard population dim P across 8 cores (2 pops/core), embarrassingly parallel.
  - Cast x/w to fp8e4m3 on host (0/1 values are exact); compute notx = 1-x on
    device (ACT/DVE); accumulate x@w0 + notx@w1 into the same PSUM bank via a
    single K=4096 "concat" contraction -> one accumulation group, no bias pass.
  - fp8 DoubleRow matmuls (K=256 per MM) for 2x PE throughput.
  - PSUM f32 accumulation of 0/1 products is exact (max 4096 < 2^24), so the
    result is bit-exact vs the f32 reference.

Host-side work is layout only: slicing, transpose, dtype cast, and the final
gather. All arithmetic (notx, matmuls) happens on device.
"""

import os

import numpy as np
import ml_dtypes

from concourse import bacc, tile, mybir
from concourse.bass_utils import run_bass_kernel_spmd

P_TOT, B, I, O = 16, 512, 2048, 2048
N_CORES = 8
PPC = P_TOT // N_CORES  # pops per core = 2
PART = 128

FP8 = mybir.dt.float8e4
F16 = mybir.dt.float16
F32 = mybir.dt.float32
NP_FP8 = ml_dtypes.float8_e4m3


def build_nc(ppc=PPC, b=B, i_dim=I, o_dim=O, n_cores=N_CORES, use_dr=True):
    """Build + compile the per-core Bass program (SPMD: same program, 8 cores)."""
    kt = i_dim // PART          # k-subtiles per weight tensor (16)
    nb = o_dim // 512           # o-blocks (4)
    mb = b // PART              # b-subtiles (4)
    DR = mybir.MatmulPerfMode.DoubleRow if use_dr else None
    kstep = 2 if use_dr else 1

    nc = bacc.Bacc("TRN2", target_bir_lowering=False, debug=False,
                   num_devices=n_cores)

    xt_d = nc.dram_tensor("xt", [ppc, PART, kt, b], FP8, kind="ExternalInput")
    w0_d = nc.dram_tensor("w0", [ppc, nb, PART, kt, 512], FP8, kind="ExternalInput")
    w1_d = nc.dram_tensor("w1", [ppc, nb, PART, kt, 512], FP8, kind="ExternalInput")
    out_d = nc.dram_tensor("out", [ppc, b, o_dim], F32, kind="ExternalOutput")

    with tile.TileContext(nc) as tc:
        with (
            tc.tile_pool(name="warm", bufs=1) as warm,
            tc.tile_pool(name="xpool", bufs=2) as xpool,
            tc.tile_pool(name="wpool", bufs=8) as wpool,
            tc.tile_pool(name="opool", bufs=4) as opool,
            tc.tile_pool(name="pspool", bufs=4, space="PSUM") as pspool,
            tc.tile_pool(name="warmps", bufs=1, space="PSUM") as warmps,
        ):
            for pop in range(ppc):
                xt = xpool.tile([PART, kt, b], FP8, tag="xt")
                nxt = xpool.tile([PART, kt, b], FP8, tag="nxt")
                # x chunked on the scalar ring ahead of w1: the first matmul
                # needs only xt[:, 0:2, :], so a 256KB first chunk unblocks
                # the first LDWEIGHTS ~10us sooner than one 1MB transfer.
                xch = min(4, kt)
                for ch in range(0, kt, xch):
                    nc.scalar.dma_start(out=xt[:, ch:ch + xch, :],
                                        in_=xt_d.ap()[pop, :, ch:ch + xch, :])
                    # notx = 1 - x  ==  (x * -1) + 1, per chunk
                    nc.vector.tensor_scalar(
                        nxt[:, ch:ch + xch, :], xt[:, ch:ch + xch, :], -1.0, 1.0,
                        mybir.AluOpType.mult, mybir.AluOpType.add,
                    )
                for nbi in range(nb):
                    w0t = wpool.tile([PART, kt, 512], FP8, tag="w")
                    w1t = wpool.tile([PART, kt, 512], FP8, tag="w")
                    # w0 loads on the sync HWDGE ring, w1 on the scalar HWDGE
                    # ring (output stores go via gpsimd/SWDGE) so stores never
                    # block weight prefetch in a shared FIFO. Chunked k-wise so
                    # the first matmuls start before the whole block lands; the
                    # very first block uses finer chunks to cut the startup
                    # bubble before the first LDWEIGHTS.
                    wch = 2 if (pop == 0 and nbi == 0) else 4
                    for ch in range(0, kt, wch):
                        nc.sync.dma_start(
                            out=w0t[:, ch:ch + wch, :],
                            in_=w0_d.ap()[pop, nbi, :, ch:ch + wch, :])
                        nc.scalar.dma_start(
                            out=w1t[:, ch:ch + wch, :],
                            in_=w1_d.ap()[pop, nbi, :, ch:ch + wch, :])
                    for m in range(mb):
                        ps = pspool.tile([PART, 512], F32)
                        msl = slice(m * PART, (m + 1) * PART)
                        nk = kt // kstep
                        for kd in range(nk):
                            ksl = slice(kd * kstep, (kd + 1) * kstep)
                            nc.tensor.matmul(
                                ps[:], lhsT=xt[:, ksl, msl], rhs=w0t[:, ksl, :],
                                start=(kd == 0), stop=False, perf_mode=DR,
                            )
                        for kd in range(nk):
                            ksl = slice(kd * kstep, (kd + 1) * kstep)
                            nc.tensor.matmul(
                                ps[:], lhsT=nxt[:, ksl, msl], rhs=w1t[:, ksl, :],
                                start=False, stop=(kd == nk - 1), perf_mode=DR,
                            )
                        ot = opool.tile([PART, 512], F32)
                        nc.vector.tensor_copy(ot[:], ps[:])
                        nc.gpsimd.dma_start(
                            out=out_d.ap()[pop, msl, nbi * 512:(nbi + 1) * 512],
                            in_=ot[:],
                        )
    nc.compile()
    return nc


def build_nc_v3(ppc=PPC, b=B, i_dim=I, o_dim=O, n_cores=N_CORES):
    """v3: concat scheme (as v1) with stationary reuse.

    All weights for one population stay SBUF-resident (8MB fp8); the matmul
    loop is m -> half -> kd -> nb so one LDWEIGHTS serves 4 matmuls (one per
    o-block), cutting LDW traffic 4x and keeping the PE stream dense. PSUM
    holds 4 accumulating banks (one per o-block) per m-subtile.
    """
    kt = i_dim // PART
    nb = o_dim // 512
    mb = b // PART
    DR = mybir.MatmulPerfMode.DoubleRow
    nk = kt // 2

    nc = bacc.Bacc("TRN2", target_bir_lowering=False, debug=False,
                   num_devices=n_cores)

    xt_d = nc.dram_tensor("xt", [ppc, PART, kt, b], FP8, kind="ExternalInput")
    w0_d = nc.dram_tensor("w0", [ppc, nb, PART, kt, 512], FP8, kind="ExternalInput")
    w1_d = nc.dram_tensor("w1", [ppc, nb, PART, kt, 512], FP8, kind="ExternalInput")
    out_d = nc.dram_tensor("out", [ppc, b, o_dim], F32, kind="ExternalOutput")

    with tile.TileContext(nc) as tc:
        with (
            tc.tile_pool(name="xpool", bufs=2) as xpool,
            tc.tile_pool(name="wpool", bufs=2 * nb * 2) as wpool,
            tc.tile_pool(name="opool", bufs=6) as opool,
            tc.tile_pool(name="pspool", bufs=8, space="PSUM") as pspool,
        ):
            for pop in range(ppc):
                xt = xpool.tile([PART, kt, b], FP8, tag="xt")
                nxt = xpool.tile([PART, kt, b], FP8, tag="nxt")
                nc.gpsimd.dma_start(out=xt[:], in_=xt_d.ap()[pop])
                nc.vector.tensor_scalar(
                    nxt[:], xt[:], -1.0, 1.0,
                    mybir.AluOpType.mult, mybir.AluOpType.add,
                )
                # all weights for this pop, k-chunked so matmuls start early;
                # w0 on the sync HWDGE ring, w1 on the scalar HWDGE ring
                w0t = [wpool.tile([PART, kt, 512], FP8, tag="w",
                                  name=f"w0t_{pop}_{i}") for i in range(nb)]
                w1t = [wpool.tile([PART, kt, 512], FP8, tag="w",
                                  name=f"w1t_{pop}_{i}") for i in range(nb)]
                for ch in range(0, kt, 4):
                    for nbi in range(nb):
                        nc.sync.dma_start(
                            out=w0t[nbi][:, ch:ch + 4, :],
                            in_=w0_d.ap()[pop, nbi, :, ch:ch + 4, :])
                        nc.scalar.dma_start(
                            out=w1t[nbi][:, ch:ch + 4, :],
                            in_=w1_d.ap()[pop, nbi, :, ch:ch + 4, :])
                for m in range(mb):
                    msl = slice(m * PART, (m + 1) * PART)
                    pss = [pspool.tile([PART, 512], F32, tag="ps",
                                       name=f"ps_{pop}_{m}_{i}") for i in range(nb)]
                    for half, (xsrc, wt) in enumerate(((xt, w0t), (nxt, w1t))):
                        for kd in range(nk):
                            ksl = slice(2 * kd, 2 * kd + 2)
                            for nbi in range(nb):
                                nc.tensor.matmul(
                                    pss[nbi][:], lhsT=xsrc[:, ksl, msl],
                                    rhs=wt[nbi][:, ksl, :],
                                    start=(half == 0 and kd == 0),
                                    stop=(half == 1 and kd == nk - 1),
                                    perf_mode=DR,
                                )
                    for nbi in range(nb):
                        ot = opool.tile([PART, 512], F32)
                        nc.vector.tensor_copy(ot[:], pss[nbi][:])
                        nc.gpsimd.dma_start(
                            out=out_d.ap()[pop, msl, nbi * 512:(nbi + 1) * 512],
                            in_=ot[:],
                        )
    nc.compile()
    return nc


def build_nc_v4(ppc=PPC, b=B, i_dim=I, o_dim=O, n_cores=N_CORES):
    """v4: out = x@(w0-w1) + colsum(w1), wd built by DVE+gpsimd tensor_tensor.

    Halves the PE matmul stream vs the concat scheme (K=2048 instead of 4096).
    Per o-block: load w0/w1, bias = colsum(w1) via an all-ones DR matmul,
    wd = w0-w1 with the k-subtiles split between vector (11) and gpsimd (5)
    engines, main matmuls accumulate x@wd, and the DVE evacuation adds bias
    (tensor_tensor add against a bias tile copied from the bias PSUM bank).
    """
    kt = i_dim // PART
    nb = o_dim // 512
    mb = b // PART
    DR = mybir.MatmulPerfMode.DoubleRow
    nk = kt // 2
    # all subtract work on DVE: offloading 2 k-subtiles to gpsimd measured
    # 128.6us vs 128.0us all-DVE — the DVE's 23us of idle means it is not
    # strictly binding, and the gpsimd offload does not pay
    kdve = kt

    nc = bacc.Bacc("TRN2", target_bir_lowering=False, debug=False,
                   num_devices=n_cores)

    xt_d = nc.dram_tensor("xt", [ppc, PART, kt, b], FP8, kind="ExternalInput")
    w0_d = nc.dram_tensor("w0", [ppc, nb, PART, kt, 512], FP8, kind="ExternalInput")
    w1_d = nc.dram_tensor("w1", [ppc, nb, PART, kt, 512], FP8, kind="ExternalInput")
    out_d = nc.dram_tensor("out", [ppc, b, o_dim], F32, kind="ExternalOutput")

    with tile.TileContext(nc) as tc:
        with (
            tc.tile_pool(name="const", bufs=1) as const,
            tc.tile_pool(name="xpool", bufs=2) as xpool,
            tc.tile_pool(name="wsrc", bufs=6) as wsrc,
            tc.tile_pool(name="wdpool", bufs=4) as wdpool,
            tc.tile_pool(name="bpool", bufs=3) as bpool,
            tc.tile_pool(name="opool", bufs=4) as opool,
            tc.tile_pool(name="pspool", bufs=4, space="PSUM") as pspool,
            tc.tile_pool(name="psbias", bufs=2, space="PSUM") as psbias,
        ):
            ones = const.tile([PART, 2, PART], FP8)
            nc.vector.memset(ones[:], 1.0)
            xts = {}
            state = {}
            blocks = [(pop, nbi) for pop in range(ppc) for nbi in range(nb)]

            def prepare(pop, nbi):
                if nbi == 0:
                    xt = xpool.tile([PART, kt, b], FP8, tag="xt",
                                    name=f"xt_{pop}")
                    xch = min(4, kt)
                    for ch in range(0, kt, xch):
                        nc.scalar.dma_start(
                            out=xt[:, ch:ch + xch, :],
                            in_=xt_d.ap()[pop, :, ch:ch + xch, :])
                    xts[pop] = xt
                w0t = wsrc.tile([PART, kt, 512], FP8, tag="ws",
                                name=f"w0t_{pop}_{nbi}")
                w1t = wsrc.tile([PART, kt, 512], FP8, tag="ws",
                                name=f"w1t_{pop}_{nbi}")
                wch = 2 if (pop == 0 and nbi == 0) else 4
                for ch in range(0, kt, wch):
                    nc.sync.dma_start(
                        out=w1t[:, ch:ch + wch, :],
                        in_=w1_d.ap()[pop, nbi, :, ch:ch + wch, :])
                    nc.scalar.dma_start(
                        out=w0t[:, ch:ch + wch, :],
                        in_=w0_d.ap()[pop, nbi, :, ch:ch + wch, :])
                # bias = colsum(w1) (all rows of psb identical)
                psb = psbias.tile([PART, 512], F32, tag="psb")
                for kd in range(nk):
                    ksl = slice(2 * kd, 2 * kd + 2)
                    nc.tensor.matmul(
                        psb[:], lhsT=ones[:], rhs=w1t[:, ksl, :],
                        start=(kd == 0), stop=(kd == nk - 1), perf_mode=DR)
                bias_sb = bpool.tile([PART, 512], F32, tag="bias")
                nc.vector.tensor_copy(bias_sb[:], psb[:])
                # wd = w0 - w1 on DVE in fine k-chunks; emitted one block
                # AHEAD of the consuming matmuls (software pipeline) so these
                # sit before the previous block's evacuations in the DVE FIFO
                wd = wdpool.tile([PART, kt, 512], FP8, tag="wd")
                sch = max(1, kt // 8)
                for ch in range(0, kdve, sch):
                    nc.vector.tensor_tensor(
                        wd[:, ch:ch + sch, :], w0t[:, ch:ch + sch, :],
                        w1t[:, ch:ch + sch, :], mybir.AluOpType.subtract)
                if kdve < kt:
                    nc.gpsimd.tensor_tensor(
                        wd[:, kdve:, :], w0t[:, kdve:, :], w1t[:, kdve:, :],
                        mybir.AluOpType.subtract)
                state[(pop, nbi)] = (wd, bias_sb)

            def main(pop, nbi):
                wd, bias_sb = state.pop((pop, nbi))
                xt = xts[pop]
                for m in range(mb):
                    ps = pspool.tile([PART, 512], F32, tag="ps",
                                     name=f"ps_{pop}_{nbi}_{m}")
                    msl = slice(m * PART, (m + 1) * PART)
                    for kd in range(nk):
                        ksl = slice(2 * kd, 2 * kd + 2)
                        nc.tensor.matmul(
                            ps[:], lhsT=xt[:, ksl, msl], rhs=wd[:, ksl, :],
                            start=(kd == 0), stop=(kd == nk - 1), perf_mode=DR)
                    ot = opool.tile([PART, 512], F32, tag="ot",
                                    name=f"ot_{pop}_{nbi}_{m}")
                    nc.vector.tensor_tensor(
                        ot[:], ps[:], bias_sb[:], mybir.AluOpType.add)
                    nc.gpsimd.dma_start(
                        out=out_d.ap()[pop, msl, nbi * 512:(nbi + 1) * 512],
                        in_=ot[:])

            for i in range(len(blocks) + 1):
                if i < len(blocks):
                    prepare(*blocks[i])
                if i > 0:
                    main(*blocks[i - 1])
    nc.compile()
    return nc


def build_nc_v5(ppc=PPC, b=B, i_dim=I, o_dim=O, n_cores=N_CORES, look=2):
    """v5: v2's DMA-accum wd trick + v4's software pipelining + fp16 output.

    Per (pop, nbi) block:
      prepare: load -w1 (sync HWDGE) into a 544-strided tile (512B SBUF write
               runs keep the accum ucode legal); -bias = colsum(-w1) via ones
               DR matmuls (PE); bias copied psum->SBUF on ACT; wd = w0 + (-w1)
               via the SWDGE inline-ALU accum DMA — zero compute-engine cost.
      main:    4 m-subtiles x 8 DR matmuls accumulate x@wd in PSUM; DVE
               evacuates psum - (-bias) straight to fp16; scalar-ring DMA
               stores fp16 to DRAM (half the store traffic; all values are
               integers <= 2048, fp16-exact).
    prepare runs `look` blocks ahead of main so the serial w1-load -> bias-MM
    -> accum-DMA chain hides under earlier blocks' main matmuls and the PE
    never idles long enough to re-throttle (HAM).
    """
    kt = i_dim // PART          # 16
    nb = o_dim // 512           # 4
    mb = b // PART              # 4
    DR = mybir.MatmulPerfMode.DoubleRow
    nk = kt // 2                # 8

    nc = bacc.Bacc("TRN2", target_bir_lowering=False, debug=False,
                   num_devices=n_cores)

    xt_d = nc.dram_tensor("xt", [ppc, PART, kt, b], FP8, kind="ExternalInput")
    w0_d = nc.dram_tensor("w0", [ppc, nb, PART, kt, 512], FP8, kind="ExternalInput")
    w1_d = nc.dram_tensor("w1", [ppc, nb, PART, kt, 512], FP8, kind="ExternalInput")
    out_d = nc.dram_tensor("out", [ppc, b, o_dim], F16, kind="ExternalOutput")

    with tile.TileContext(nc) as tc:
        with (
            tc.tile_pool(name="const", bufs=1) as const,
            tc.tile_pool(name="xpool", bufs=2) as xpool,
            tc.tile_pool(name="wpool", bufs=look + 2) as wpool,
            tc.tile_pool(name="bpool", bufs=look + 2) as bpool,
            tc.tile_pool(name="opool", bufs=6) as opool,
            tc.tile_pool(name="pspool", bufs=4, space="PSUM") as pspool,
            tc.tile_pool(name="psbias", bufs=3, space="PSUM") as psbias,
        ):
            ones = const.tile([PART, 2, PART], FP8)
            nc.vector.memset(ones[:], 1.0)
            xts = {}
            state = {}
            blocks = [(pop, nbi) for pop in range(ppc) for nbi in range(nb)]

            def prepare(i):
                pop, nbi = blocks[i]
                if nbi == 0:
                    xt = xpool.tile([PART, kt, b], FP8, tag="xt",
                                    name=f"xt_{pop}")
                    xch = 4 if pop == 0 else kt
                    for ch in range(0, kt, xch):
                        nc.scalar.dma_start(out=xt[:, ch:ch + xch, :],
                                            in_=xt_d.ap()[pop, :, ch:ch + xch, :])
                    xts[pop] = xt
                wdp = wpool.tile([PART, kt, 544], FP8, tag="w", name=f"wd_{i}")
                wd = wdp[:, :, :512]
                wch = 4 if i == 0 else 8
                for ch in range(0, kt, wch):
                    nc.sync.dma_start(out=wd[:, ch:ch + wch, :],
                                      in_=w1_d.ap()[pop, nbi, :, ch:ch + wch, :])
                psb = psbias.tile([PART, 512], F32, tag="psb", name=f"psb_{i}")
                for kd in range(nk):
                    ksl = slice(2 * kd, 2 * kd + 2)
                    nc.tensor.matmul(psb[:], lhsT=ones[:], rhs=wd[:, ksl, :],
                                     start=(kd == 0), stop=(kd == nk - 1),
                                     perf_mode=DR)
                bias_sb = bpool.tile([PART, 512], F32, tag="bias",
                                     name=f"bias_{i}")
                nc.scalar.copy(bias_sb[:], psb[:])
                nc.gpsimd.dma_start(out=wd[:], in_=w0_d.ap()[pop, nbi],
                                    accum_op=mybir.AluOpType.add)
                state[i] = (wd, bias_sb)

            def main(i):
                pop, nbi = blocks[i]
                wd, bias_sb = state.pop(i)
                xt = xts[pop]
                for m in range(mb):
                    ps = pspool.tile([PART, 512], F32, tag="ps",
                                     name=f"ps_{i}_{m}")
                    msl = slice(m * PART, (m + 1) * PART)
                    for kd in range(nk):
                        ksl = slice(2 * kd, 2 * kd + 2)
                        nc.tensor.matmul(ps[:], lhsT=xt[:, ksl, msl],
                                         rhs=wd[:, ksl, :],
                                         start=(kd == 0), stop=(kd == nk - 1),
                                         perf_mode=DR)
                    ot = opool.tile([PART, 512], F16, tag="ot",
                                    name=f"ot_{i}_{m}")
                    # out = psum - (-bias), evacuated straight to fp16
                    nc.vector.tensor_tensor(ot[:], ps[:], bias_sb[:],
                                            mybir.AluOpType.subtract)
                    nc.scalar.dma_start(
                        out=out_d.ap()[pop, msl, nbi * 512:(nbi + 1) * 512],
                        in_=ot[:])

            n = len(blocks)
            for i in range(n + look):
                if i < n:
                    prepare(i)
                if i >= look:
                    main(i - look)
    nc.compile()
    return nc


def build_nc_v6(ppc=PPC, b=B, i_dim=I, o_dim=O, n_cores=N_CORES, look=2):
    """v6: XOR-as-subtract. wd = w0 - w1 computed as a bitwise XOR on int32
    words (4 fp8 bytes per DVE lane-cycle -> 4x fewer DVE elements than v4's
    fp8 tensor_tensor subtract).

    Host encodes w1 as (0.0 - w1): bytes {0x00, 0xB8} = {+0.0, -1.0}; w0 is
    standard fp8 {0x00, 0x38}. XOR gives {0x00,0x38,0xB8,0x80} =
    {+0,+1,-1,-0} == w0-w1 exactly (the -0.0 is additive identity in the PE).

    Per block: dense 512KB HWDGE loads of w0/w1 as int32; bias = colsum(-w1)
    via ones DR matmuls on the fp8 bitcast view; ACT copies bias psum->SBUF;
    one DVE XOR (FD=2048 int32, ~2.2us) makes wd; 32 DR main matmuls; DVE
    evacuates psum-(-bias) to an [128, 2048] fp16 out tile, stored once per
    (pop, m) as a dense 512KB DMA. prepare() runs `look` blocks ahead.
    """
    kt = i_dim // PART          # 16
    nb = o_dim // 512           # 4
    mb = b // PART              # 4
    wi = 512 // 4               # int32 words per 512 fp8
    DR = mybir.MatmulPerfMode.DoubleRow
    nk = kt // 2                # 8

    nc = bacc.Bacc("TRN2", target_bir_lowering=False, debug=False,
                   num_devices=n_cores)

    I32 = mybir.dt.int32
    xt_d = nc.dram_tensor("xt", [ppc, PART, kt, b], FP8, kind="ExternalInput")
    w0_d = nc.dram_tensor("w0", [ppc, nb, PART, kt, wi], I32, kind="ExternalInput")
    w1_d = nc.dram_tensor("w1", [ppc, nb, PART, kt, wi], I32, kind="ExternalInput")
    out_d = nc.dram_tensor("out", [ppc, b, o_dim], F16, kind="ExternalOutput")

    with tile.TileContext(nc) as tc:
        with (
            tc.tile_pool(name="const", bufs=1) as const,
            tc.tile_pool(name="xpool", bufs=2) as xpool,
            tc.tile_pool(name="wsrc", bufs=2 * (look + 2)) as wsrc,
            tc.tile_pool(name="wdpool", bufs=look + 2) as wdpool,
            tc.tile_pool(name="bpool", bufs=look + 2) as bpool,
            tc.tile_pool(name="opool", bufs=2 * mb) as opool,
            tc.tile_pool(name="pspool", bufs=4, space="PSUM") as pspool,
            tc.tile_pool(name="psbias", bufs=3, space="PSUM") as psbias,
        ):
            ones = const.tile([PART, 2, PART], FP8)
            nc.vector.memset(ones[:], 1.0)
            xts = {}
            ots = {}
            state = {}
            blocks = [(pop, nbi) for pop in range(ppc) for nbi in range(nb)]

            def prepare(i):
                pop, nbi = blocks[i]
                if nbi == 0:
                    xt = xpool.tile([PART, kt, b], FP8, tag="xt",
                                    name=f"xt_{pop}")
                    xch = 4 if pop == 0 else kt
                    for ch in range(0, kt, xch):
                        nc.scalar.dma_start(out=xt[:, ch:ch + xch, :],
                                            in_=xt_d.ap()[pop, :, ch:ch + xch, :])
                    xts[pop] = xt
                w0t = wsrc.tile([PART, kt, wi], I32, tag="ws",
                                name=f"w0t_{i}")
                w1t = wsrc.tile([PART, kt, wi], I32, tag="ws",
                                name=f"w1t_{i}")
                wch = 8 if i == 0 else kt
                for ch in range(0, kt, wch):
                    nc.sync.dma_start(out=w1t[:, ch:ch + wch, :],
                                      in_=w1_d.ap()[pop, nbi, :, ch:ch + wch, :])
                for ch in range(0, kt, wch):
                    nc.sync.dma_start(out=w0t[:, ch:ch + wch, :],
                                      in_=w0_d.ap()[pop, nbi, :, ch:ch + wch, :])
                # -bias = colsum(-w1) off the fp8 view of the int32 tile
                psb = psbias.tile([PART, 512], F32, tag="psb", name=f"psb_{i}")
                for kd in range(nk):
                    ksl = slice(2 * kd, 2 * kd + 2)
                    nc.tensor.matmul(psb[:], lhsT=ones[:],
                                     rhs=w1t[:, ksl, :].bitcast(FP8),
                                     start=(kd == 0), stop=(kd == nk - 1),
                                     perf_mode=DR)
                bias_sb = bpool.tile([PART, 512], F32, tag="bias",
                                     name=f"bias_{i}")
                nc.scalar.copy(bias_sb[:], psb[:])
                # wd = w0 - w1 == w0 XOR enc(w1), 4 bytes per lane-cycle
                wd = wdpool.tile([PART, kt, wi], I32, tag="wd", name=f"wd_{i}")
                nc.vector.tensor_tensor(wd[:], w0t[:], w1t[:],
                                        mybir.AluOpType.bitwise_xor)
                state[i] = (wd, bias_sb)

            def main(i):
                pop, nbi = blocks[i]
                wd, bias_sb = state.pop(i)
                xt = xts[pop]
                for m in range(mb):
                    if nbi == 0:
                        ots[(pop, m)] = opool.tile([PART, o_dim], F16, tag="ot",
                                                   name=f"ot_{pop}_{m}")
                    ps = pspool.tile([PART, 512], F32, tag="ps",
                                     name=f"ps_{i}_{m}")
                    msl = slice(m * PART, (m + 1) * PART)
                    for kd in range(nk):
                        ksl = slice(2 * kd, 2 * kd + 2)
                        nc.tensor.matmul(ps[:], lhsT=xt[:, ksl, msl],
                                         rhs=wd[:, ksl, :].bitcast(FP8),
                                         start=(kd == 0), stop=(kd == nk - 1),
                                         perf_mode=DR)
                    ot = ots[(pop, m)]
                    # out = psum - (-bias), straight to fp16
                    nc.vector.tensor_tensor(ot[:, nbi * 512:(nbi + 1) * 512],
                                            ps[:], bias_sb[:],
                                            mybir.AluOpType.subtract)
                    if nbi == nb - 1:
                        nc.scalar.dma_start(out=out_d.ap()[pop, msl, :],
                                            in_=ot[:])

            n = len(blocks)
            for i in range(n + look):
                if i < n:
                    prepare(i)
                if i >= look:
                    main(i - look)
    nc.compile()
    return nc


def build_nc_v2(ppc=PPC, b=B, i_dim=I, o_dim=O, n_cores=N_CORES):
    """v2: algebraic rewrite out = x@(w0-w1) + colsum(w1).

    The w1 input tensor holds -w1 (sign applied during the host fp8 cast;
    walrus rejects cce_op=subtract but accepts add):
    - wd = w0 + (-w1) computed by the gpsimd DMA inline ALU (accum_op=add)
      while loading w0 — zero compute-engine cost.
    - colsum(-w1) = -bias via an all-ones stationary matmul against the tile
      while it still holds -w1, once per o-block.
    - main pass: psum = x @ wd, half the PE work of v1; evacuated as
      psum - (-bias) with a DVE tensor_tensor subtract.
    All values stay exact: x in {0,1}, wd in {-1,0,1} (fp8 exact), bias and
    accumulation in f32 (integers < 2^24).
    """
    kt = i_dim // PART
    nb = o_dim // 512
    mb = b // PART
    DR = mybir.MatmulPerfMode.DoubleRow
    nk = kt // 2

    nc = bacc.Bacc("TRN2", target_bir_lowering=False, debug=False,
                   num_devices=n_cores)

    xt_d = nc.dram_tensor("xt", [ppc, PART, kt, b], FP8, kind="ExternalInput")
    w0_d = nc.dram_tensor("w0", [ppc, nb, PART, kt, 512], FP8, kind="ExternalInput")
    w1_d = nc.dram_tensor("w1", [ppc, nb, PART, kt, 512], FP8, kind="ExternalInput")
    out_d = nc.dram_tensor("out", [ppc, b, o_dim], F32, kind="ExternalOutput")

    with tile.TileContext(nc) as tc:
        with (
            tc.tile_pool(name="const", bufs=1) as const,
            tc.tile_pool(name="xpool", bufs=2) as xpool,
            tc.tile_pool(name="wpool", bufs=4) as wpool,
            tc.tile_pool(name="bpool", bufs=2) as bpool,
            tc.tile_pool(name="opool", bufs=4) as opool,
            tc.tile_pool(name="pspool", bufs=4, space="PSUM") as pspool,
            tc.tile_pool(name="psbias", bufs=2, space="PSUM") as psbias,
        ):
            ones = const.tile([PART, 2, PART], FP8)
            nc.vector.memset(ones[:], 1.0)
            for pop in range(ppc):
                xt = xpool.tile([PART, kt, b], FP8, tag="xt")
                nc.scalar.dma_start(out=xt[:], in_=xt_d.ap()[pop])
                for nbi in range(nb):
                    # 544-wide rows (512 data + 32 pad): keeps every SBUF write
                    # run at 512B so the accum DMA's RMW ucode accepts it (runs
                    # >512B crash the exec unit), and stops the AP optimizer
                    # from merging rows into one big run.
                    wdp = wpool.tile([PART, kt, 544], FP8, tag="w")
                    wd = wdp[:, :, :512]
                    # 1) load -w1 (sync HWDGE ring)
                    wch = min(8, kt)
                    for ch in range(0, kt, wch):
                        nc.sync.dma_start(
                            out=wd[:, ch:ch + wch, :],
                            in_=w1_d.ap()[pop, nbi, :, ch:ch + wch, :])
                    # 2) -bias = colsum(-w1) while the tile still holds -w1
                    psb = psbias.tile([PART, 512], F32)
                    for kd in range(nk):
                        ksl = slice(2 * kd, 2 * kd + 2)
                        nc.tensor.matmul(
                            psb[:], lhsT=ones[:], rhs=wd[:, ksl, :],
                            start=(kd == 0), stop=(kd == nk - 1), perf_mode=DR)
                    bias_sb = bpool.tile([PART, 512], F32, tag="bias")
                    nc.vector.tensor_copy(bias_sb[:], psb[:])
                    # 3) wd = w0 + (-w1) via DMA inline ALU (op(in,out) = in+out)
                    nc.gpsimd.dma_start(out=wd[:], in_=w0_d.ap()[pop, nbi],
                                        accum_op=mybir.AluOpType.add)
                    # 4) main pass: psum = x @ wd, evac with bias add
                    for m in range(mb):
                        ps = pspool.tile([PART, 512], F32)
                        msl = slice(m * PART, (m + 1) * PART)
                        for kd in range(nk):
                            ksl = slice(2 * kd, 2 * kd + 2)
                            nc.tensor.matmul(
                                ps[:], lhsT=xt[:, ksl, msl], rhs=wd[:, ksl, :],
                                start=(kd == 0), stop=(kd == nk - 1), perf_mode=DR)
                        ot = opool.tile([PART, 512], F32)
                        # out = psum - (-bias)
                        nc.vector.tensor_tensor(
                            ot[:], ps[:], bias_sb[:], mybir.AluOpType.subtract)
                        nc.scalar.dma_start(
                            out=out_d.ap()[pop, msl, nbi * 512:(nbi + 1) * 512],
                            in_=ot[:])
    nc.compile()
    return nc


def prep_core_inputs(x, w, core, ppc=PPC, negate_w1=None):
    """Layout-only host prep for one core: slice pops, transpose x, tile, cast.
    With negate_w1, the fp8 cast of w1 carries a sign flip (v2/v5 send -w1 so
    the device can form w0-w1 with the DMA ALU's accum add). Defaults from
    K_VERSION so test harnesses that call this without the flag stay correct."""
    if negate_w1 is None:
        negate_w1 = K_VERSION in (2, 5)
    if K_VERSION == 6:
        return prep_core_inputs_v6(x, w, core, ppc)
    p0 = core * ppc
    b, i_dim = x.shape[1], x.shape[2]
    o_dim = w.shape[4]
    kt = i_dim // PART
    nb = o_dim // 512
    xs = x[p0:p0 + ppc]                       # [ppc, B, I]
    # xT partition-tiled: [ppc, 128, kt, B];  xt[p, kp, kti, b] = x[p, b, kti*128+kp]
    xt = np.ascontiguousarray(
        xs.reshape(ppc, b, kt, PART).transpose(0, 3, 2, 1)
    ).astype(NP_FP8)
    ws = w[:, p0:p0 + ppc, 0]                 # [2, ppc, I, O]
    # [2, ppc, nb, 128, kt, 512]; wt[j,p,nbi,kp,kti,no] = w[j,p,kti*128+kp, nbi*512+no]
    wt = np.ascontiguousarray(
        ws.reshape(2, ppc, kt, PART, nb, 512).transpose(0, 1, 4, 3, 2, 5)
    )
    w0 = wt[0].astype(NP_FP8)
    w1 = (-wt[1]).astype(NP_FP8) if negate_w1 else wt[1].astype(NP_FP8)
    return {"xt": xt, "w0": w0, "w1": w1}


def prep_core_inputs_v6(x, w, core, ppc=PPC):
    """Host prep for v6: same layout as v2/v4, but the weight tensors are
    shipped as int32 views of the fp8 bytes (so the device can XOR them), and
    w1 is encoded as (0.0 - w1) -> bytes {0x00, 0xB8} with a positive zero."""
    p0 = core * ppc
    b, i_dim = x.shape[1], x.shape[2]
    o_dim = w.shape[4]
    kt = i_dim // PART
    nb = o_dim // 512
    xs = x[p0:p0 + ppc]
    xt = np.ascontiguousarray(
        xs.reshape(ppc, b, kt, PART).transpose(0, 3, 2, 1)
    ).astype(NP_FP8)
    ws = w[:, p0:p0 + ppc, 0]
    wt = np.ascontiguousarray(
        ws.reshape(2, ppc, kt, PART, nb, 512).transpose(0, 1, 4, 3, 2, 5)
    )
    w0 = np.ascontiguousarray(wt[0].astype(NP_FP8)).view(np.int32)
    w1 = np.ascontiguousarray(
        (np.float32(0.0) - wt[1]).astype(NP_FP8)).view(np.int32)
    return {"xt": xt, "w0": w0, "w1": w1}


_NC_CACHE = {}

# which builder kernel() uses: 1 = concat (x@w0 + notx@w1), 2 = DMA-subtract
# trick, 4 = DVE-subtract pipelined, 5 = DMA-subtract pipelined + fp16 out,
# 6 = XOR-as-subtract + fp16 out
K_VERSION = int(os.environ.get("EVO_KERNEL_VERSION", "6"))


def _get_nc():
    if "nc" not in _NC_CACHE:
        builder = {1: build_nc, 2: build_nc_v2, 3: build_nc_v3,
                   4: build_nc_v4, 5: build_nc_v5, 6: build_nc_v6}[K_VERSION]
        _NC_CACHE["nc"] = builder()
    return _NC_CACHE["nc"]


def kernel(x, w):
    x = np.asarray(x)
    w = np.asarray(w)
    nc = _get_nc()
    in_maps = [prep_core_inputs(x, w, c) for c in range(N_CORES)]
    res = run_bass_kernel_spmd(nc, in_maps, list(range(N_CORES)))
    out = np.concatenate([res.results[c]["out"] for c in range(N_CORES)], axis=0)
    return np.ascontiguousarray(out.astype(np.float32))



# revision 16
# speedup vs baseline: 1.6599x; 1.0620x over previous
"""Bass/Trainium2 kernel for nn_EvoBinarizedLayer.

Reference computation (P=16 populations, B=512, I=O=2048, all values 0/1):
    out[p,b,o] = sum_i x[p,b,i]*w0[p,i,o] + (1-x[p,b,i])*w1[p,i,o]

Strategy:
  - Shard population dim P across 8 cores (2 pops/core), embarrassingly parallel.
  - Cast x/w to fp8e4m3 on host (0/1 values are exact); compute notx = 1-x on
    device (ACT/DVE); accumulate x@w0 + notx@w1 into the same PSUM bank via a
    single K=4096 "concat" contraction -> one accumulation group, no bias pass.
  - fp8 DoubleRow matmuls (K=256 per MM) for 2x PE throughput.
  - PSUM f32 accumulation of 0/1 products is exact (max 4096 < 2^24), so the
    result is bit-exact vs the f32 reference.

Host-side work is layout only: slicing, transpose, dtype cast, and the final
gather. All arithmetic (notx, matmuls) happens on device.
"""

import os

import numpy as np
import ml_dtypes

from concourse import bacc, tile, mybir
from concourse.bass_utils import run_bass_kernel_spmd

P_TOT, B, I, O = 16, 512, 2048, 2048
N_CORES = 8
PPC = P_TOT // N_CORES  # pops per core = 2
PART = 128

FP8 = mybir.dt.float8e4
F16 = mybir.dt.float16
F32 = mybir.dt.float32
NP_FP8 = ml_dtypes.float8_e4m3


def build_nc(ppc=PPC, b=B, i_dim=I, o_dim=O, n_cores=N_CORES, use_dr=True):
    """Build + compile the per-core Bass program (SPMD: same program, 8 cores)."""
    kt = i_dim // PART          # k-subtiles per weight tensor (16)
    nb = o_dim // 512           # o-blocks (4)
    mb = b // PART              # b-subtiles (4)
    DR = mybir.MatmulPerfMode.DoubleRow if use_dr else None
    kstep = 2 if use_dr else 1

    nc = bacc.Bacc("TRN2", target_bir_lowering=False, debug=False,
                   num_devices=n_cores)

    xt_d = nc.dram_tensor("xt", [ppc, PART, kt, b], FP8, kind="ExternalInput")
    w0_d = nc.dram_tensor("w0", [ppc, nb, PART, kt, 512], FP8, kind="ExternalInput")
    w1_d = nc.dram_tensor("w1", [ppc, nb, PART, kt, 512], FP8, kind="ExternalInput")
    out_d = nc.dram_tensor("out", [ppc, b, o_dim], F32, kind="ExternalOutput")

    with tile.TileContext(nc) as tc:
        with (
            tc.tile_pool(name="warm", bufs=1) as warm,
            tc.tile_pool(name="xpool", bufs=2) as xpool,
            tc.tile_pool(name="wpool", bufs=8) as wpool,
            tc.tile_pool(name="opool", bufs=4) as opool,
            tc.tile_pool(name="pspool", bufs=4, space="PSUM") as pspool,
            tc.tile_pool(name="warmps", bufs=1, space="PSUM") as warmps,
        ):
            for pop in range(ppc):
                xt = xpool.tile([PART, kt, b], FP8, tag="xt")
                nxt = xpool.tile([PART, kt, b], FP8, tag="nxt")
                # x chunked on the scalar ring ahead of w1: the first matmul
                # needs only xt[:, 0:2, :], so a 256KB first chunk unblocks
                # the first LDWEIGHTS ~10us sooner than one 1MB transfer.
                xch = min(4, kt)
                for ch in range(0, kt, xch):
                    nc.scalar.dma_start(out=xt[:, ch:ch + xch, :],
                                        in_=xt_d.ap()[pop, :, ch:ch + xch, :])
                    # notx = 1 - x  ==  (x * -1) + 1, per chunk
                    nc.vector.tensor_scalar(
                        nxt[:, ch:ch + xch, :], xt[:, ch:ch + xch, :], -1.0, 1.0,
                        mybir.AluOpType.mult, mybir.AluOpType.add,
                    )
                for nbi in range(nb):
                    w0t = wpool.tile([PART, kt, 512], FP8, tag="w")
                    w1t = wpool.tile([PART, kt, 512], FP8, tag="w")
                    # w0 loads on the sync HWDGE ring, w1 on the scalar HWDGE
                    # ring (output stores go via gpsimd/SWDGE) so stores never
                    # block weight prefetch in a shared FIFO. Chunked k-wise so
                    # the first matmuls start before the whole block lands; the
                    # very first block uses finer chunks to cut the startup
                    # bubble before the first LDWEIGHTS.
                    wch = 2 if (pop == 0 and nbi == 0) else 4
                    for ch in range(0, kt, wch):
                        nc.sync.dma_start(
                            out=w0t[:, ch:ch + wch, :],
                            in_=w0_d.ap()[pop, nbi, :, ch:ch + wch, :])
                        nc.scalar.dma_start(
                            out=w1t[:, ch:ch + wch, :],
                            in_=w1_d.ap()[pop, nbi, :, ch:ch + wch, :])
                    for m in range(mb):
                        ps = pspool.tile([PART, 512], F32)
                        msl = slice(m * PART, (m + 1) * PART)
                        nk = kt // kstep
                        for kd in range(nk):
                            ksl = slice(kd * kstep, (kd + 1) * kstep)
                            nc.tensor.matmul(
                                ps[:], lhsT=xt[:, ksl, msl], rhs=w0t[:, ksl, :],
                                start=(kd == 0), stop=False, perf_mode=DR,
                            )
                        for kd in range(nk):
                            ksl = slice(kd * kstep, (kd + 1) * kstep)
                            nc.tensor.matmul(
                                ps[:], lhsT=nxt[:, ksl, msl], rhs=w1t[:, ksl, :],
                                start=False, stop=(kd == nk - 1), perf_mode=DR,
                            )
                        ot = opool.tile([PART, 512], F32)
                        nc.vector.tensor_copy(ot[:], ps[:])
                        nc.gpsimd.dma_start(
                            out=out_d.ap()[pop, msl, nbi * 512:(nbi + 1) * 512],
                            in_=ot[:],
                        )
    nc.compile()
    return nc


def build_nc_v3(ppc=PPC, b=B, i_dim=I, o_dim=O, n_cores=N_CORES):
    """v3: concat scheme (as v1) with stationary reuse.

    All weights for one population stay SBUF-resident (8MB fp8); the matmul
    loop is m -> half -> kd -> nb so one LDWEIGHTS serves 4 matmuls (one per
    o-block), cutting LDW traffic 4x and keeping the PE stream dense. PSUM
    holds 4 accumulating banks (one per o-block) per m-subtile.
    """
    kt = i_dim // PART
    nb = o_dim // 512
    mb = b // PART
    DR = mybir.MatmulPerfMode.DoubleRow
    nk = kt // 2

    nc = bacc.Bacc("TRN2", target_bir_lowering=False, debug=False,
                   num_devices=n_cores)

    xt_d = nc.dram_tensor("xt", [ppc, PART, kt, b], FP8, kind="ExternalInput")
    w0_d = nc.dram_tensor("w0", [ppc, nb, PART, kt, 512], FP8, kind="ExternalInput")
    w1_d = nc.dram_tensor("w1", [ppc, nb, PART, kt, 512], FP8, kind="ExternalInput")
    out_d = nc.dram_tensor("out", [ppc, b, o_dim], F32, kind="ExternalOutput")

    with tile.TileContext(nc) as tc:
        with (
            tc.tile_pool(name="xpool", bufs=2) as xpool,
            tc.tile_pool(name="wpool", bufs=2 * nb * 2) as wpool,
            tc.tile_pool(name="opool", bufs=6) as opool,
            tc.tile_pool(name="pspool", bufs=8, space="PSUM") as pspool,
        ):
            for pop in range(ppc):
                xt = xpool.tile([PART, kt, b], FP8, tag="xt")
                nxt = xpool.tile([PART, kt, b], FP8, tag="nxt")
                nc.gpsimd.dma_start(out=xt[:], in_=xt_d.ap()[pop])
                nc.vector.tensor_scalar(
                    nxt[:], xt[:], -1.0, 1.0,
                    mybir.AluOpType.mult, mybir.AluOpType.add,
                )
                # all weights for this pop, k-chunked so matmuls start early;
                # w0 on the sync HWDGE ring, w1 on the scalar HWDGE ring
                w0t = [wpool.tile([PART, kt, 512], FP8, tag="w",
                                  name=f"w0t_{pop}_{i}") for i in range(nb)]
                w1t = [wpool.tile([PART, kt, 512], FP8, tag="w",
                                  name=f"w1t_{pop}_{i}") for i in range(nb)]
                for ch in range(0, kt, 4):
                    for nbi in range(nb):
                        nc.sync.dma_start(
                            out=w0t[nbi][:, ch:ch + 4, :],
                            in_=w0_d.ap()[pop, nbi, :, ch:ch + 4, :])
                        nc.scalar.dma_start(
                            out=w1t[nbi][:, ch:ch + 4, :],
                            in_=w1_d.ap()[pop, nbi, :, ch:ch + 4, :])
                for m in range(mb):
                    msl = slice(m * PART, (m + 1) * PART)
                    pss = [pspool.tile([PART, 512], F32, tag="ps",
                                       name=f"ps_{pop}_{m}_{i}") for i in range(nb)]
                    for half, (xsrc, wt) in enumerate(((xt, w0t), (nxt, w1t))):
                        for kd in range(nk):
                            ksl = slice(2 * kd, 2 * kd + 2)
                            for nbi in range(nb):
                                nc.tensor.matmul(
                                    pss[nbi][:], lhsT=xsrc[:, ksl, msl],
                                    rhs=wt[nbi][:, ksl, :],
                                    start=(half == 0 and kd == 0),
                                    stop=(half == 1 and kd == nk - 1),
                                    perf_mode=DR,
                                )
                    for nbi in range(nb):
                        ot = opool.tile([PART, 512], F32)
                        nc.vector.tensor_copy(ot[:], pss[nbi][:])
                        nc.gpsimd.dma_start(
                            out=out_d.ap()[pop, msl, nbi * 512:(nbi + 1) * 512],
                            in_=ot[:],
                        )
    nc.compile()
    return nc


def build_nc_v4(ppc=PPC, b=B, i_dim=I, o_dim=O, n_cores=N_CORES):
    """v4: out = x@(w0-w1) + colsum(w1), wd built by DVE+gpsimd tensor_tensor.

    Halves the PE matmul stream vs the concat scheme (K=2048 instead of 4096).
    Per o-block: load w0/w1, bias = colsum(w1) via an all-ones DR matmul,
    wd = w0-w1 with the k-subtiles split between vector (11) and gpsimd (5)
    engines, main matmuls accumulate x@wd, and the DVE evacuation adds bias
    (tensor_tensor add against a bias tile copied from the bias PSUM bank).
    """
    kt = i_dim // PART
    nb = o_dim // 512
    mb = b // PART
    DR = mybir.MatmulPerfMode.DoubleRow
    nk = kt // 2
    # all subtract work on DVE: offloading 2 k-subtiles to gpsimd measured
    # 128.6us vs 128.0us all-DVE — the DVE's 23us of idle means it is not
    # strictly binding, and the gpsimd offload does not pay
    kdve = kt

    nc = bacc.Bacc("TRN2", target_bir_lowering=False, debug=False,
                   num_devices=n_cores)

    xt_d = nc.dram_tensor("xt", [ppc, PART, kt, b], FP8, kind="ExternalInput")
    w0_d = nc.dram_tensor("w0", [ppc, nb, PART, kt, 512], FP8, kind="ExternalInput")
    w1_d = nc.dram_tensor("w1", [ppc, nb, PART, kt, 512], FP8, kind="ExternalInput")
    out_d = nc.dram_tensor("out", [ppc, b, o_dim], F32, kind="ExternalOutput")

    with tile.TileContext(nc) as tc:
        with (
            tc.tile_pool(name="const", bufs=1) as const,
            tc.tile_pool(name="xpool", bufs=2) as xpool,
            tc.tile_pool(name="wsrc", bufs=6) as wsrc,
            tc.tile_pool(name="wdpool", bufs=4) as wdpool,
            tc.tile_pool(name="bpool", bufs=3) as bpool,
            tc.tile_pool(name="opool", bufs=4) as opool,
            tc.tile_pool(name="pspool", bufs=4, space="PSUM") as pspool,
            tc.tile_pool(name="psbias", bufs=2, space="PSUM") as psbias,
        ):
            ones = const.tile([PART, 2, PART], FP8)
            nc.vector.memset(ones[:], 1.0)
            xts = {}
            state = {}
            blocks = [(pop, nbi) for pop in range(ppc) for nbi in range(nb)]

            def prepare(pop, nbi):
                if nbi == 0:
                    xt = xpool.tile([PART, kt, b], FP8, tag="xt",
                                    name=f"xt_{pop}")
                    xch = min(4, kt)
                    for ch in range(0, kt, xch):
                        nc.scalar.dma_start(
                            out=xt[:, ch:ch + xch, :],
                            in_=xt_d.ap()[pop, :, ch:ch + xch, :])
                    xts[pop] = xt
                w0t = wsrc.tile([PART, kt, 512], FP8, tag="ws",
                                name=f"w0t_{pop}_{nbi}")
                w1t = wsrc.tile([PART, kt, 512], FP8, tag="ws",
                                name=f"w1t_{pop}_{nbi}")
                wch = 2 if (pop == 0 and nbi == 0) else 4
                for ch in range(0, kt, wch):
                    nc.sync.dma_start(
                        out=w1t[:, ch:ch + wch, :],
                        in_=w1_d.ap()[pop, nbi, :, ch:ch + wch, :])
                    nc.scalar.dma_start(
                        out=w0t[:, ch:ch + wch, :],
                        in_=w0_d.ap()[pop, nbi, :, ch:ch + wch, :])
                # bias = colsum(w1) (all rows of psb identical)
                psb = psbias.tile([PART, 512], F32, tag="psb")
                for kd in range(nk):
                    ksl = slice(2 * kd, 2 * kd + 2)
                    nc.tensor.matmul(
                        psb[:], lhsT=ones[:], rhs=w1t[:, ksl, :],
                        start=(kd == 0), stop=(kd == nk - 1), perf_mode=DR)
                bias_sb = bpool.tile([PART, 512], F32, tag="bias")
                nc.vector.tensor_copy(bias_sb[:], psb[:])
                # wd = w0 - w1 on DVE in fine k-chunks; emitted one block
                # AHEAD of the consuming matmuls (software pipeline) so these
                # sit before the previous block's evacuations in the DVE FIFO
                wd = wdpool.tile([PART, kt, 512], FP8, tag="wd")
                sch = max(1, kt // 8)
                for ch in range(0, kdve, sch):
                    nc.vector.tensor_tensor(
                        wd[:, ch:ch + sch, :], w0t[:, ch:ch + sch, :],
                        w1t[:, ch:ch + sch, :], mybir.AluOpType.subtract)
                if kdve < kt:
                    nc.gpsimd.tensor_tensor(
                        wd[:, kdve:, :], w0t[:, kdve:, :], w1t[:, kdve:, :],
                        mybir.AluOpType.subtract)
                state[(pop, nbi)] = (wd, bias_sb)

            def main(pop, nbi):
                wd, bias_sb = state.pop((pop, nbi))
                xt = xts[pop]
                for m in range(mb):
                    ps = pspool.tile([PART, 512], F32, tag="ps",
                                     name=f"ps_{pop}_{nbi}_{m}")
                    msl = slice(m * PART, (m + 1) * PART)
                    for kd in range(nk):
                        ksl = slice(2 * kd, 2 * kd + 2)
                        nc.tensor.matmul(
                            ps[:], lhsT=xt[:, ksl, msl], rhs=wd[:, ksl, :],
                            start=(kd == 0), stop=(kd == nk - 1), perf_mode=DR)
                    ot = opool.tile([PART, 512], F32, tag="ot",
                                    name=f"ot_{pop}_{nbi}_{m}")
                    nc.vector.tensor_tensor(
                        ot[:], ps[:], bias_sb[:], mybir.AluOpType.add)
                    nc.gpsimd.dma_start(
                        out=out_d.ap()[pop, msl, nbi * 512:(nbi + 1) * 512],
                        in_=ot[:])

            for i in range(len(blocks) + 1):
                if i < len(blocks):
                    prepare(*blocks[i])
                if i > 0:
                    main(*blocks[i - 1])
    nc.compile()
    return nc


def build_nc_v5(ppc=PPC, b=B, i_dim=I, o_dim=O, n_cores=N_CORES, look=2):
    """v5: v2's DMA-accum wd trick + v4's software pipelining + fp16 output.

    Per (pop, nbi) block:
      prepare: load -w1 (sync HWDGE) into a 544-strided tile (512B SBUF write
               runs keep the accum ucode legal); -bias = colsum(-w1) via ones
               DR matmuls (PE); bias copied psum->SBUF on ACT; wd = w0 + (-w1)
               via the SWDGE inline-ALU accum DMA — zero compute-engine cost.
      main:    4 m-subtiles x 8 DR matmuls accumulate x@wd in PSUM; DVE
               evacuates psum - (-bias) straight to fp16; scalar-ring DMA
               stores fp16 to DRAM (half the store traffic; all values are
               integers <= 2048, fp16-exact).
    prepare runs `look` blocks ahead of main so the serial w1-load -> bias-MM
    -> accum-DMA chain hides under earlier blocks' main matmuls and the PE
    never idles long enough to re-throttle (HAM).
    """
    kt = i_dim // PART          # 16
    nb = o_dim // 512           # 4
    mb = b // PART              # 4
    DR = mybir.MatmulPerfMode.DoubleRow
    nk = kt // 2                # 8

    nc = bacc.Bacc("TRN2", target_bir_lowering=False, debug=False,
                   num_devices=n_cores)

    xt_d = nc.dram_tensor("xt", [ppc, PART, kt, b], FP8, kind="ExternalInput")
    w0_d = nc.dram_tensor("w0", [ppc, nb, PART, kt, 512], FP8, kind="ExternalInput")
    w1_d = nc.dram_tensor("w1", [ppc, nb, PART, kt, 512], FP8, kind="ExternalInput")
    out_d = nc.dram_tensor("out", [ppc, b, o_dim], F16, kind="ExternalOutput")

    with tile.TileContext(nc) as tc:
        with (
            tc.tile_pool(name="const", bufs=1) as const,
            tc.tile_pool(name="xpool", bufs=2) as xpool,
            tc.tile_pool(name="wpool", bufs=look + 2) as wpool,
            tc.tile_pool(name="bpool", bufs=look + 2) as bpool,
            tc.tile_pool(name="opool", bufs=6) as opool,
            tc.tile_pool(name="pspool", bufs=4, space="PSUM") as pspool,
            tc.tile_pool(name="psbias", bufs=3, space="PSUM") as psbias,
        ):
            ones = const.tile([PART, 2, PART], FP8)
            nc.vector.memset(ones[:], 1.0)
            xts = {}
            state = {}
            blocks = [(pop, nbi) for pop in range(ppc) for nbi in range(nb)]

            def prepare(i):
                pop, nbi = blocks[i]
                if nbi == 0:
                    xt = xpool.tile([PART, kt, b], FP8, tag="xt",
                                    name=f"xt_{pop}")
                    xch = 4 if pop == 0 else kt
                    for ch in range(0, kt, xch):
                        nc.scalar.dma_start(out=xt[:, ch:ch + xch, :],
                                            in_=xt_d.ap()[pop, :, ch:ch + xch, :])
                    xts[pop] = xt
                wdp = wpool.tile([PART, kt, 544], FP8, tag="w", name=f"wd_{i}")
                wd = wdp[:, :, :512]
                wch = 4 if i == 0 else 8
                for ch in range(0, kt, wch):
                    nc.sync.dma_start(out=wd[:, ch:ch + wch, :],
                                      in_=w1_d.ap()[pop, nbi, :, ch:ch + wch, :])
                psb = psbias.tile([PART, 512], F32, tag="psb", name=f"psb_{i}")
                for kd in range(nk):
                    ksl = slice(2 * kd, 2 * kd + 2)
                    nc.tensor.matmul(psb[:], lhsT=ones[:], rhs=wd[:, ksl, :],
                                     start=(kd == 0), stop=(kd == nk - 1),
                                     perf_mode=DR)
                bias_sb = bpool.tile([PART, 512], F32, tag="bias",
                                     name=f"bias_{i}")
                nc.scalar.copy(bias_sb[:], psb[:])
                nc.gpsimd.dma_start(out=wd[:], in_=w0_d.ap()[pop, nbi],
                                    accum_op=mybir.AluOpType.add)
                state[i] = (wd, bias_sb)

            def main(i):
                pop, nbi = blocks[i]
                wd, bias_sb = state.pop(i)
                xt = xts[pop]
                for m in range(mb):
                    ps = pspool.tile([PART, 512], F32, tag="ps",
                                     name=f"ps_{i}_{m}")
                    msl = slice(m * PART, (m + 1) * PART)
                    for kd in range(nk):
                        ksl = slice(2 * kd, 2 * kd + 2)
                        nc.tensor.matmul(ps[:], lhsT=xt[:, ksl, msl],
                                         rhs=wd[:, ksl, :],
                                         start=(kd == 0), stop=(kd == nk - 1),
                                         perf_mode=DR)
                    ot = opool.tile([PART, 512], F16, tag="ot",
                                    name=f"ot_{i}_{m}")
                    # out = psum - (-bias), evacuated straight to fp16
                    nc.vector.tensor_tensor(ot[:], ps[:], bias_sb[:],
                                            mybir.AluOpType.subtract)
                    nc.scalar.dma_start(
                        out=out_d.ap()[pop, msl, nbi * 512:(nbi + 1) * 512],
                        in_=ot[:])

            n = len(blocks)
            for i in range(n + look):
                if i < n:
                    prepare(i)
                if i >= look:
                    main(i - look)
    nc.compile()
    return nc


def build_nc_v6(ppc=PPC, b=B, i_dim=I, o_dim=O, n_cores=N_CORES, look=2):
    """v6: XOR-as-subtract. wd = w0 - w1 computed as a bitwise XOR on int32
    words (4 fp8 bytes per DVE lane-cycle -> 4x fewer DVE elements than v4's
    fp8 tensor_tensor subtract).

    Host encodes w1 as (0.0 - w1): bytes {0x00, 0xB8} = {+0.0, -1.0}; w0 is
    standard fp8 {0x00, 0x38}. XOR gives {0x00,0x38,0xB8,0x80} =
    {+0,+1,-1,-0} == w0-w1 exactly (the -0.0 is additive identity in the PE).

    Per block: dense 512KB HWDGE loads of w0/w1 as int32; bias = colsum(-w1)
    via ones DR matmuls on the fp8 bitcast view; ACT copies bias psum->SBUF;
    one DVE XOR (FD=2048 int32, ~2.2us) makes wd; 32 DR main matmuls; DVE
    evacuates psum-(-bias) to an [128, 2048] fp16 out tile, stored once per
    (pop, m) as a dense 512KB DMA. prepare() runs `look` blocks ahead.
    """
    kt = i_dim // PART          # 16
    nb = o_dim // 512           # 4
    mb = b // PART              # 4
    wi = 512 // 4               # int32 words per 512 fp8
    DR = mybir.MatmulPerfMode.DoubleRow
    nk = kt // 2                # 8

    nc = bacc.Bacc("TRN2", target_bir_lowering=False, debug=False,
                   num_devices=n_cores)

    I32 = mybir.dt.int32
    xt_d = nc.dram_tensor("xt", [ppc, PART, kt, b], FP8, kind="ExternalInput")
    w0_d = nc.dram_tensor("w0", [ppc, nb, PART, kt, wi], I32, kind="ExternalInput")
    w1_d = nc.dram_tensor("w1", [ppc, nb, PART, kt, wi], I32, kind="ExternalInput")
    out_d = nc.dram_tensor("out", [ppc, b, o_dim], F16, kind="ExternalOutput")

    with tile.TileContext(nc) as tc:
        with (
            tc.tile_pool(name="const", bufs=1) as const,
            tc.tile_pool(name="xpool", bufs=2) as xpool,
            tc.tile_pool(name="wsrc", bufs=2 * (look + 2)) as wsrc,
            tc.tile_pool(name="wdpool", bufs=look + 2) as wdpool,
            tc.tile_pool(name="bpool", bufs=look + 2) as bpool,
            tc.tile_pool(name="opool", bufs=2 * mb) as opool,
            tc.tile_pool(name="pspool", bufs=4, space="PSUM") as pspool,
            tc.tile_pool(name="psbias", bufs=3, space="PSUM") as psbias,
        ):
            ones = const.tile([PART, 2, PART], FP8)
            nc.vector.memset(ones[:], 1.0)
            xts = {}
            ots = {}
            state = {}
            blocks = [(pop, nbi) for pop in range(ppc) for nbi in range(nb)]

            def load(i):
                pop, nbi = blocks[i]
                if nbi == 0:
                    xt = xpool.tile([PART, kt, b], FP8, tag="xt",
                                    name=f"xt_{pop}")
                    xch = 4 if pop == 0 else kt
                    for ch in range(0, kt, xch):
                        nc.scalar.dma_start(out=xt[:, ch:ch + xch, :],
                                            in_=xt_d.ap()[pop, :, ch:ch + xch, :])
                    xts[pop] = xt
                w0t = wsrc.tile([PART, kt, wi], I32, tag="ws",
                                name=f"w0t_{i}")
                w1t = wsrc.tile([PART, kt, wi], I32, tag="ws",
                                name=f"w1t_{i}")
                # block 0 loads in fine chunks so the first bias matmul (and
                # the PE warm-up) starts as early as possible
                wch = 2 if i == 0 else kt
                for ch in range(0, kt, wch):
                    nc.sync.dma_start(out=w1t[:, ch:ch + wch, :],
                                      in_=w1_d.ap()[pop, nbi, :, ch:ch + wch, :])
                for ch in range(0, kt, wch):
                    nc.scalar.dma_start(out=w0t[:, ch:ch + wch, :],
                                        in_=w0_d.ap()[pop, nbi, :, ch:ch + wch, :])
                state[i] = (w0t, w1t)

            def finish(i):
                pop, nbi = blocks[i]
                w0t, w1t = state.pop(i)
                # -bias = colsum(-w1) off the fp8 view of the int32 tile
                psb = psbias.tile([PART, 512], F32, tag="psb", name=f"psb_{i}")
                for kd in range(nk):
                    ksl = slice(2 * kd, 2 * kd + 2)
                    nc.tensor.matmul(psb[:], lhsT=ones[:],
                                     rhs=w1t[:, ksl, :].bitcast(FP8),
                                     start=(kd == 0), stop=(kd == nk - 1),
                                     perf_mode=DR)
                bias_sb = bpool.tile([PART, 512], F32, tag="bias",
                                     name=f"bias_{i}")
                nc.scalar.copy(bias_sb[:], psb[:])
                # wd = w0 - w1 == w0 XOR enc(w1), 4 bytes per DVE lane-cycle
                # (bitwise ops are DVE-only). Emitted AFTER main(i-look)'s
                # evacuations so a late weight load can't stall psum reuse
                # through the DVE FIFO.
                wd = wdpool.tile([PART, kt, wi], I32, tag="wd", name=f"wd_{i}")
                xch = kt // 2 if i < 2 else kt
                for ch in range(0, kt, xch):
                    nc.vector.tensor_tensor(
                        wd[:, ch:ch + xch, :], w0t[:, ch:ch + xch, :],
                        w1t[:, ch:ch + xch, :], mybir.AluOpType.bitwise_xor)
                state[("wd", i)] = (wd, bias_sb)

            def main(i):
                pop, nbi = blocks[i]
                wd, bias_sb = state.pop(("wd", i))
                xt = xts[pop]
                for m in range(mb):
                    if nbi == 0:
                        ots[(pop, m)] = opool.tile([PART, o_dim], F16, tag="ot",
                                                   name=f"ot_{pop}_{m}")
                    ps = pspool.tile([PART, 512], F32, tag="ps",
                                     name=f"ps_{i}_{m}")
                    msl = slice(m * PART, (m + 1) * PART)
                    for kd in range(nk):
                        ksl = slice(2 * kd, 2 * kd + 2)
                        nc.tensor.matmul(ps[:], lhsT=xt[:, ksl, msl],
                                         rhs=wd[:, ksl, :].bitcast(FP8),
                                         start=(kd == 0), stop=(kd == nk - 1),
                                         perf_mode=DR)
                    ot = ots[(pop, m)]
                    # out = psum - (-bias), straight to fp16
                    nc.vector.tensor_tensor(ot[:, nbi * 512:(nbi + 1) * 512],
                                            ps[:], bias_sb[:],
                                            mybir.AluOpType.subtract)
                    # store in halves (after nbi 1 and 3) so the final store
                    # tail is only 256KB instead of 2MB
                    if nbi == 1:
                        nc.scalar.dma_start(out=out_d.ap()[pop, msl, 0:1024],
                                            in_=ot[:, 0:1024])
                    elif nbi == nb - 1:
                        nc.scalar.dma_start(out=out_d.ap()[pop, msl, 1024:2048],
                                            in_=ot[:, 1024:2048])

            n = len(blocks)
            for i in range(n + look):
                if i < n:
                    load(i)
                if i >= look:
                    main(i - look)
                if i < n:
                    finish(i)
    nc.compile()
    return nc


def build_nc_v2(ppc=PPC, b=B, i_dim=I, o_dim=O, n_cores=N_CORES):
    """v2: algebraic rewrite out = x@(w0-w1) + colsum(w1).

    The w1 input tensor holds -w1 (sign applied during the host fp8 cast;
    walrus rejects cce_op=subtract but accepts add):
    - wd = w0 + (-w1) computed by the gpsimd DMA inline ALU (accum_op=add)
      while loading w0 — zero compute-engine cost.
    - colsum(-w1) = -bias via an all-ones stationary matmul against the tile
      while it still holds -w1, once per o-block.
    - main pass: psum = x @ wd, half the PE work of v1; evacuated as
      psum - (-bias) with a DVE tensor_tensor subtract.
    All values stay exact: x in {0,1}, wd in {-1,0,1} (fp8 exact), bias and
    accumulation in f32 (integers < 2^24).
    """
    kt = i_dim // PART
    nb = o_dim // 512
    mb = b // PART
    DR = mybir.MatmulPerfMode.DoubleRow
    nk = kt // 2

    nc = bacc.Bacc("TRN2", target_bir_lowering=False, debug=False,
                   num_devices=n_cores)

    xt_d = nc.dram_tensor("xt", [ppc, PART, kt, b], FP8, kind="ExternalInput")
    w0_d = nc.dram_tensor("w0", [ppc, nb, PART, kt, 512], FP8, kind="ExternalInput")
    w1_d = nc.dram_tensor("w1", [ppc, nb, PART, kt, 512], FP8, kind="ExternalInput")
    out_d = nc.dram_tensor("out", [ppc, b, o_dim], F32, kind="ExternalOutput")

    with tile.TileContext(nc) as tc:
        with (
            tc.tile_pool(name="const", bufs=1) as const,
            tc.tile_pool(name="xpool", bufs=2) as xpool,
            tc.tile_pool(name="wpool", bufs=4) as wpool,
            tc.tile_pool(name="bpool", bufs=2) as bpool,
            tc.tile_pool(name="opool", bufs=4) as opool,
            tc.tile_pool(name="pspool", bufs=4, space="PSUM") as pspool,
            tc.tile_pool(name="psbias", bufs=2, space="PSUM") as psbias,
        ):
            ones = const.tile([PART, 2, PART], FP8)
            nc.vector.memset(ones[:], 1.0)
            for pop in range(ppc):
                xt = xpool.tile([PART, kt, b], FP8, tag="xt")
                nc.scalar.dma_start(out=xt[:], in_=xt_d.ap()[pop])
                for nbi in range(nb):
                    # 544-wide rows (512 data + 32 pad): keeps every SBUF write
                    # run at 512B so the accum DMA's RMW ucode accepts it (runs
                    # >512B crash the exec unit), and stops the AP optimizer
                    # from merging rows into one big run.
                    wdp = wpool.tile([PART, kt, 544], FP8, tag="w")
                    wd = wdp[:, :, :512]
                    # 1) load -w1 (sync HWDGE ring)
                    wch = min(8, kt)
                    for ch in range(0, kt, wch):
                        nc.sync.dma_start(
                            out=wd[:, ch:ch + wch, :],
                            in_=w1_d.ap()[pop, nbi, :, ch:ch + wch, :])
                    # 2) -bias = colsum(-w1) while the tile still holds -w1
                    psb = psbias.tile([PART, 512], F32)
                    for kd in range(nk):
                        ksl = slice(2 * kd, 2 * kd + 2)
                        nc.tensor.matmul(
                            psb[:], lhsT=ones[:], rhs=wd[:, ksl, :],
                            start=(kd == 0), stop=(kd == nk - 1), perf_mode=DR)
                    bias_sb = bpool.tile([PART, 512], F32, tag="bias")
                    nc.vector.tensor_copy(bias_sb[:], psb[:])
                    # 3) wd = w0 + (-w1) via DMA inline ALU (op(in,out) = in+out)
                    nc.gpsimd.dma_start(out=wd[:], in_=w0_d.ap()[pop, nbi],
                                        accum_op=mybir.AluOpType.add)
                    # 4) main pass: psum = x @ wd, evac with bias add
                    for m in range(mb):
                        ps = pspool.tile([PART, 512], F32)
                        msl = slice(m * PART, (m + 1) * PART)
                        for kd in range(nk):
                            ksl = slice(2 * kd, 2 * kd + 2)
                            nc.tensor.matmul(
                                ps[:], lhsT=xt[:, ksl, msl], rhs=wd[:, ksl, :],
                                start=(kd == 0), stop=(kd == nk - 1), perf_mode=DR)
                        ot = opool.tile([PART, 512], F32)
                        # out = psum - (-bias)
                        nc.vector.tensor_tensor(
                            ot[:], ps[:], bias_sb[:], mybir.AluOpType.subtract)
                        nc.scalar.dma_start(
                            out=out_d.ap()[pop, msl, nbi * 512:(nbi + 1) * 512],
                            in_=ot[:])
    nc.compile()
    return nc


def prep_core_inputs(x, w, core, ppc=PPC, negate_w1=None):
    """Layout-only host prep for one core: slice pops, transpose x, tile, cast.
    With negate_w1, the fp8 cast of w1 carries a sign flip (v2/v5 send -w1 so
    the device can form w0-w1 with the DMA ALU's accum add). Defaults from
    K_VERSION so test harnesses that call this without the flag stay correct."""
    if negate_w1 is None:
        negate_w1 = K_VERSION in (2, 5)
    if K_VERSION == 6:
        return prep_core_inputs_v6(x, w, core, ppc)
    p0 = core * ppc
    b, i_dim = x.shape[1], x.shape[2]
    o_dim = w.shape[4]
    kt = i_dim // PART
    nb = o_dim // 512
    xs = x[p0:p0 + ppc]                       # [ppc, B, I]
    # xT partition-tiled: [ppc, 128, kt, B];  xt[p, kp, kti, b] = x[p, b, kti*128+kp]
    xt = np.ascontiguousarray(
        xs.reshape(ppc, b, kt, PART).transpose(0, 3, 2, 1)
    ).astype(NP_FP8)
    ws = w[:, p0:p0 + ppc, 0]                 # [2, ppc, I, O]
    # [2, ppc, nb, 128, kt, 512]; wt[j,p,nbi,kp,kti,no] = w[j,p,kti*128+kp, nbi*512+no]
    wt = np.ascontiguousarray(
        ws.reshape(2, ppc, kt, PART, nb, 512).transpose(0, 1, 4, 3, 2, 5)
    )
    w0 = wt[0].astype(NP_FP8)
    w1 = (-wt[1]).astype(NP_FP8) if negate_w1 else wt[1].astype(NP_FP8)
    return {"xt": xt, "w0": w0, "w1": w1}


def prep_core_inputs_v6(x, w, core, ppc=PPC):
    """Host prep for v6: same layout as v2/v4, but the weight tensors are
    shipped as int32 views of the fp8 bytes (so the device can XOR them), and
    w1 is encoded as (0.0 - w1) -> bytes {0x00, 0xB8} with a positive zero."""
    p0 = core * ppc
    b, i_dim = x.shape[1], x.shape[2]
    o_dim = w.shape[4]
    kt = i_dim // PART
    nb = o_dim // 512
    xs = x[p0:p0 + ppc]
    xt = np.ascontiguousarray(
        xs.reshape(ppc, b, kt, PART).transpose(0, 3, 2, 1)
    ).astype(NP_FP8)
    ws = w[:, p0:p0 + ppc, 0]
    wt = np.ascontiguousarray(
        ws.reshape(2, ppc, kt, PART, nb, 512).transpose(0, 1, 4, 3, 2, 5)
    )
    w0 = np.ascontiguousarray(wt[0].astype(NP_FP8)).view(np.int32)
    w1 = np.ascontiguousarray(
        (np.float32(0.0) - wt[1]).astype(NP_FP8)).view(np.int32)
    return {"xt": xt, "w0": w0, "w1": w1}


_NC_CACHE = {}

# which builder kernel() uses: 1 = concat (x@w0 + notx@w1), 2 = DMA-subtract
# trick, 4 = DVE-subtract pipelined, 5 = DMA-subtract pipelined + fp16 out,
# 6 = XOR-as-subtract + fp16 out
K_VERSION = int(os.environ.get("EVO_KERNEL_VERSION", "6"))


def _get_nc():
    if "nc" not in _NC_CACHE:
        builder = {1: build_nc, 2: build_nc_v2, 3: build_nc_v3,
                   4: build_nc_v4, 5: build_nc_v5, 6: build_nc_v6}[K_VERSION]
        _NC_CACHE["nc"] = builder()
    return _NC_CACHE["nc"]


def kernel(x, w):
    x = np.asarray(x)
    w = np.asarray(w)
    nc = _get_nc()
    in_maps = [prep_core_inputs(x, w, c) for c in range(N_CORES)]
    res = run_bass_kernel_spmd(nc, in_maps, list(range(N_CORES)))
    out = np.concatenate([res.results[c]["out"] for c in range(N_CORES)], axis=0)
    return np.ascontiguousarray(out.astype(np.float32))

